# revision 54
# baseline (speedup 1.0000x reference)
"""Trainium2 Bass kernel for nn_AttentionBlock (B=4, H=W=64, C=64, GroupNorm(8) +
full spatial self-attention), distributed over 8 NeuronCores.

Sharding: core i handles batch b=i//2 and query-half h=i%2 (2048 of the 4096
spatial positions). Each core computes the full GroupNorm and K/V for its
image (cheap) and attention only for its query half. No collectives.

v2 pipeline:
- exp split across THREE engines: ACT (table exp) for some score pairs,
  Pool (gpsimd) and DVE for the rest via a single-op int16 Schraudolph
  (i16 = s*23.083 + 16256.5 truncated, bitcast bf16 ~= e^(s/8), max rel err
  ~4%, final output err ~3e-3; denominator uses the same approximated
  weights so softmax normalization stays consistent).
- PE stream is gap-free: warmup matmuls ramp the clock during GroupNorm
  stats, then per tile scores-pair p / attnV pair p-3 alternate, with
  qk/v production and prev-tile finish matmuls slotted into the bubbles.
- biases: bq/bk folded into the q/k PSUM->SBUF copies (per-partition add);
  bv folded into bo on the host (bv @ Wo + bo).
- softmax denominators ride as a 65th ones-column of V; reciprocal via
  the fast custom-DVE op on [1,512] then broadcast by a bf16 PE matmul.
"""

import sys

sys.path.insert(0, "/opt/trn_rl_repo")

import numpy as np

import concourse.bacc as bacc
import concourse.tile as tile
from concourse import mybir

B, H, W, C = 4, 64, 64, 64
HW = H * W  # 4096
HALF = HW // 2  # 2048
EPS = 1e-5
SCALE = C ** -0.5

F32 = mybir.dt.float32
MDT = mybir.dt.bfloat16  # PE matmul operand dtype (scores/projections)
I16 = mybir.dt.int16
I8 = mybir.dt.int8
F8E5 = mybir.dt.float8e5  # attn weights (e5m2: range to 57344 covers e^9)
F8E4 = mybir.dt.float8e4  # v values (e4m3)

# Schraudolph exp in bf16-bit space: i16 = round(s * 2^7/ln2 * SCALE + 127*2^7)
SCH_SCALE = float((2.0 ** 7) / np.log(2.0) * SCALE)
SCH_BIAS = 16251.0  # 127*2^7 shifted -5.5 to center the one-sided
# mantissa-interpolation error (+0..6.7%) around zero
# same trick in e5m2-bit space: i8 = s * 2^2/ln2 * SCALE + 15*2^2
SCH8_SCALE = float(4.0 / np.log(2.0) * SCALE)
SCH8_BIAS = 60.25

NWARM = 24  # PE warmup matmuls (ramp p-state during GN stats)
LAGS = [6, 6, 6, 3]  # attnV trails scores by LAG pairs; short last tile
# so the post-loop drain is small

# engine per exp pair: A=ACT table exp, D=DVE int16-schraudolph. (Pool cannot
# read PSUM on TRN2, so it only gets SBUF->SBUF work: xn, recb, final out.)
# Tile tails lean A so DVE is clear for the next tile's start.
EMAP0 = ['D', 'A', 'D', 'A', 'D', 'A', 'D', 'A',
         'D', 'A', 'D', 'A', 'A', 'D', 'A', 'A']      # A9 D7
EMAPN = ['D', 'A', 'D', 'A', 'D', 'A', 'D', 'A',
         'D', 'D', 'A', 'D', 'D', 'A', 'A', 'A']      # A8 D8
EMAP3 = ['D', 'A'] * 8  # strict alternation: tile 3 runs LAG=3, so each
# exp must land within ~2 pairs of its scores
EMAPS = [EMAP0, EMAPN, EMAPN, EMAP3]


def build_nc():
    nc = bacc.Bacc("TRN2", debug=False, num_devices=8)

    # ---- DRAM I/O ----
    xp_d = nc.dram_tensor("xp", [128, HALF], F32, kind="ExternalInput")
    wq_d = nc.dram_tensor("wq", [64, 128], MDT, kind="ExternalInput")
    wk_d = nc.dram_tensor("wk", [128, 128], MDT, kind="ExternalInput")
    wv_d = nc.dram_tensor("wv", [128, 128], MDT, kind="ExternalInput")
    wo_d = nc.dram_tensor("wo", [64, 64], MDT, kind="ExternalInput")
    bq_d = nc.dram_tensor("bq", [128, 1], F32, kind="ExternalInput")
    bo_d = nc.dram_tensor("bo", [128, 1], F32, kind="ExternalInput")
    gam_d = nc.dram_tensor("gam", [128, 1], F32, kind="ExternalInput")
    bet_d = nc.dram_tensor("bet", [128, 1], F32, kind="ExternalInput")
    comb_d = nc.dram_tensor("comb", [128, 128], F32, kind="ExternalInput")
    out_d = nc.dram_tensor("out", [64, HALF], F32, kind="ExternalOutput")

    with tile.TileContext(nc) as tc, \
         tc.tile_pool(name="singles", bufs=1) as singles, \
         tc.tile_pool(name="stats", bufs=1) as stats, \
         tc.tile_pool(name="sc_ps", bufs=2, space="PSUM") as sc_ps, \
         tc.tile_pool(name="pacc_ps", bufs=2, space="PSUM") as pacc_ps, \
         tc.tile_pool(name="aux_ps", bufs=1, space="PSUM") as aux_ps, \
         tc.tile_pool(name="work", bufs=2) as work:

        # ---- input DMAs: everything on the sync hwdge queue (the issuing
        # engine is otherwise idle; putting DMAs on the ACT queue would
        # block the sqrt/exp table work behind descriptor writes) ----
        x_sb = singles.tile([128, HALF], F32)
        for r in range(4):
            nc.sync.dma_start(
                x_sb[:, 512 * r: 512 * r + 512],
                xp_d.ap()[:, 512 * r: 512 * r + 512],
            )
        gam_sb = singles.tile([128, 1], F32)
        nc.sync.dma_start(gam_sb[:], gam_d.ap())
        bet_sb = singles.tile([128, 1], F32)
        nc.sync.dma_start(bet_sb[:], bet_d.ap())
        comb_sb = singles.tile([128, 128], F32)
        nc.sync.dma_start(comb_sb[:], comb_d.ap())
        wk_sb = singles.tile([128, 128], MDT)
        nc.sync.dma_start(wk_sb[:], wk_d.ap())
        wq_sb = singles.tile([64, 128], MDT)
        nc.sync.dma_start(wq_sb[:], wq_d.ap())
        bq_sb = singles.tile([128, 1], F32)
        nc.sync.dma_start(bq_sb[:], bq_d.ap())
        wv_sb = singles.tile([128, 128], MDT)
        nc.sync.dma_start(wv_sb[:], wv_d.ap())
        wo_sb = singles.tile([64, 64], MDT)
        nc.sync.dma_start(wo_sb[:], wo_d.ap())
        bo_sb = singles.tile([128, 1], F32)
        nc.sync.dma_start(bo_sb[:], bo_d.ap())

        # ---- big SBUF tensors ----
        xn_r = singles.tile([128, HALF], MDT)
        q_dup = singles.tile([128, HALF], MDT)
        kt_sb = singles.tile([128, HALF], MDT)
        # v in fp8e4m3, pair-major for DoubleRow: chunk pair p = [130p,130p+130)
        v_all = singles.tile([128, 192 * 16], F8E4)
        attnexp = singles.tile([128, 1024 * 16], F8E5)
        out_sb = singles.tile([64, HALF], F32)
        ones_sb = singles.tile([128, 512], MDT)

        # constants on Pool, first thing (warmup matmuls read ones_sb)
        nc.gpsimd.memset(ones_sb[:], 1.0)
        v4 = v_all[:].rearrange("p (t h e) -> p t h e", h=2, e=96)
        nc.gpsimd.memset(v4[:, :, :, 65:96], 0.0)
        nc.gpsimd.memset(v4[:, :, :, 64:65], 1.0)

        # pre-warm the sqrt ACT table set (used by the GN rstd); the exp
        # table is loaded right after the single sqrt below
        scr = stats.tile([128, 1], F32)
        nc.vector.memset(scr[:], 1.0)
        nc.scalar.activation(scr[:], scr[:], mybir.ActivationFunctionType.Sqrt)

        # ---- PE warmup: ramp the activity monitor while DVE does GN stats
        # (rides the sc-tag banks, which are free until the first scores) ----
        for w in range(NWARM):
            wps = sc_ps.tile([128, 512], F32, tag="sc", name=f"warm{w}")
            nc.tensor.matmul(wps[:], ones_sb[:, 0:128], ones_sb[:, :],
                             start=True, stop=True)

        # ---- GroupNorm stats: bn per partition per 512-slice, then a
        # block-diagonal averaging matmul combines across channels ----
        st6 = stats.tile([128, 4, 6], F32)
        mv4 = stats.tile([128, 4, 2], F32)
        for r in range(4):
            nc.vector.bn_stats(st6[:, r, :], x_sb[:, 512 * r: 512 * r + 512])
            nc.vector.bn_aggr(mv4[:, r, :], st6[:, r, :])
        smat = stats.tile([128, 8], F32)  # cols 0-3 mean, 4-7 E[x^2]
        nc.vector.tensor_copy(smat[:, 0:4], mv4[:, :, 0])
        nc.vector.tensor_mul(smat[:, 4:8], mv4[:, :, 0], mv4[:, :, 0])
        nc.vector.tensor_add(smat[:, 4:8], smat[:, 4:8], mv4[:, :, 1])

        cps = pacc_ps.tile([128, 8], F32, tag="pacc")
        nc.tensor.matmul(cps[:], comb_sb[:], smat[:], start=True, stop=True)
        gstat = stats.tile([128, 8], F32)  # 0-3 mean_g, 4-7 E2_g
        nc.vector.tensor_copy(gstat[:], cps[:])

        # var = E2 - mean^2; EPS folds into the sqrt's activation bias.
        # rstd = 1/sqrt(var+EPS) via ACT sqrt + fast custom-DVE reciprocal
        # (the sanctioned accurate path; far fewer serial ops than a
        # bit-trick Newton chain)
        ve = stats.tile([128, 4], F32)
        nc.vector.tensor_mul(ve[:], gstat[:, 0:4], gstat[:, 0:4])
        nc.vector.tensor_sub(ve[:], gstat[:, 4:8], ve[:])
        eps_sb = stats.tile([128, 1], F32)
        nc.vector.memset(eps_sb[:], EPS)
        sve = stats.tile([128, 4], F32)
        nc.scalar.activation(sve[:], ve[:],
                             mybir.ActivationFunctionType.Sqrt,
                             bias=eps_sb[:])
        # switch ACT to the exp table now, during idle time
        nc.scalar.activation(scr[:], scr[:], mybir.ActivationFunctionType.Exp)
        rstd = stats.tile([128, 4], F32)
        nc.vector.reciprocal_approx_fast(out=rstd[:], in_=sve[:])

        gsc = stats.tile([128, 4], F32)
        nc.vector.tensor_scalar_mul(gsc[:], rstd[:], gam_sb[:])
        gbias = stats.tile([128, 4], F32)
        nc.vector.tensor_mul(gbias[:], gstat[:, 0:4], gsc[:])
        nc.vector.tensor_scalar(
            out=gbias[:], in0=gbias[:], scalar1=-1.0, scalar2=bet_sb[:],
            op0=mybir.AluOpType.mult, op1=mybir.AluOpType.add,
        )
        # the fp32 residual pass folds in bo (bo rides rows 0:63 of the bias;
        # rows 64:127 of x_sb are never read again after the qkv matmuls)
        gbias2 = stats.tile([128, 4], F32)
        nc.vector.tensor_scalar_add(gbias2[:], gbias[:], bo_sb[:])
        # xn = x * gsc + gbias: slice 0's bf16 copy runs on DVE (it gates
        # qk0 -> first scores); the rest and the fp32 residual pass run on
        # the slow-but-idle Pool. Per slice: bf16 read first, then the
        # in-place fp32 overwrite (Pool ops ordered; DVE xn0 emitted before
        # Pool's slice-0 overwrite so the framework serializes the WAR).
        nc.vector.tensor_scalar(
            out=xn_r[:, 0:512], in0=x_sb[:, 0:512],
            scalar1=gsc[:, 0:1], scalar2=gbias[:, 0:1],
            op0=mybir.AluOpType.mult, op1=mybir.AluOpType.add,
        )
        for r in range(4):
            sl = slice(512 * r, 512 * r + 512)
            if r > 0:
                nc.gpsimd.tensor_scalar(
                    out=xn_r[:, sl], in0=x_sb[:, sl],
                    scalar1=gsc[:, r: r + 1], scalar2=gbias[:, r: r + 1],
                    op0=mybir.AluOpType.mult, op1=mybir.AluOpType.add,
                )
            nc.gpsimd.tensor_scalar(
                out=x_sb[:, sl], in0=x_sb[:, sl],
                scalar1=gsc[:, r: r + 1], scalar2=gbias2[:, r: r + 1],
                op0=mybir.AluOpType.mult, op1=mybir.AluOpType.add,
            )

        # ---- emission helpers ----
        def emit_qk_slice(t, pool_tags):
            # k^T packed by half (lhsT = blockdiag(Wk, Wk)); q^T duplicated on
            # both partition halves (lhsT = [Wq | Wq]). bk is dropped exactly
            # (a per-query score constant cancels in softmax); bq folds into
            # the q copy as a per-partition bias on ACT.
            sl = slice(512 * t, 512 * t + 512)
            pool_k, tag_k = pool_tags[0]
            pool_q, tag_q = pool_tags[1]
            ps2 = pool_k.tile([128, 512], F32, tag=tag_k, name=f"kps{t}")
            nc.tensor.matmul(ps2[:], wk_sb[:], xn_r[:, sl], start=True,
                             stop=True)
            nc.scalar.activation(kt_sb[:, sl], ps2[:],
                                 mybir.ActivationFunctionType.Identity)
            ps = pool_q.tile([128, 512], F32, tag=tag_q, name=f"qps{t}")
            nc.tensor.matmul(ps[:], wq_sb[:], xn_r[0:64, sl], start=True,
                             stop=True)
            nc.scalar.activation(
                q_dup[:, sl], ps[:], mybir.ActivationFunctionType.Identity,
                bias=bq_sb[:],
            )

        def emit_v(u, pool_tag=None):
            # v position-major, two 128-position chunks (halves) per matmul;
            # single copy with a dual-chunk strided output AP. Even u on the
            # bcq psum tag + DVE copy, odd u on fpq + ACT.
            sl = slice(128 * u, 128 * u + 128)
            pool, tag = pool_tag or (aux_ps, "bcq" if u % 2 == 0 else "fpq")
            ps = pool.tile([128, 128], F32, tag=tag, name=f"vps{u}")
            nc.tensor.matmul(ps[:], xn_r[:, sl], wv_sb[:], start=True,
                             stop=True)
            psr = ps[:].rearrange("p (h e) -> p h e", h=2)
            nc.vector.tensor_copy(v4[:, u, :, 0:64], psr[:, :, :])

        def emit_scores(n, p):
            # pair p: kv chunks p (half0, PE rows 0-63) and p+16 (half1, rows
            # 64-127) run concurrently; one [128,1024] 2-bank psum tile
            qsl = slice(512 * n, 512 * n + 512)
            ksl = slice(128 * p, 128 * p + 128)
            ps = sc_ps.tile([128, 1024], F32, tag="sc", name=f"sc{n}_{p}")
            nc.tensor.matmul(ps[:, 0:512], kt_sb[0:64, ksl],
                             q_dup[0:64, qsl], start=True, stop=True)
            nc.tensor.matmul(ps[:, 512:1024], kt_sb[64:128, ksl],
                             q_dup[64:128, qsl], start=True, stop=True)
            return ps

        def emit_exp(n, p, ps):
            # attnexp layout pair-major: chunk p at 1024p, chunk p+16 at
            # 1024p+512 — both written by this single instruction
            dst = attnexp[:, 1024 * p: 1024 * p + 1024]
            e = EMAPS[n][p]
            if e == 'A':
                nc.scalar.activation(dst, ps[:],
                                     mybir.ActivationFunctionType.Exp,
                                     scale=SCALE)
            else:
                nc.vector.tensor_scalar(
                    out=dst.bitcast(I8), in0=ps[:],
                    scalar1=SCH8_SCALE, scalar2=SCH8_BIAS,
                    op0=mybir.AluOpType.mult, op1=mybir.AluOpType.add,
                )

        paccs = {}

        def emit_attnv(n, p):
            # kv chunk pair (p, p+16) — consumes exp pair p. One fp8
            # DoubleRow matmul per pair: contraction 2x128 kv, 0.5 cyc/row.
            if n not in paccs:
                paccs[n] = pacc_ps.tile([96, 512], F32, tag="pacc",
                                        name=f"pacc{n}")
            pacc = paccs[n]
            rhs = attnexp[:, 1024 * p: 1024 * p + 1024].rearrange(
                "p (two f) -> p two f", two=2)
            nc.tensor.matmul(
                pacc[:], v4[:, p, :, :], rhs,
                start=(p == 0), stop=(p == 15),
                perf_mode=mybir.MatmulPerfMode.DoubleRow,
            )

        # finish chain for tile n, split into steps scheduled across pairs of
        # tile n+1 so the PE stream stays dense
        fin = {}

        def fin_a(n):
            # free the PSUM accumulator ASAP: unnormalized proj rows (bf16)
            # on ACT; the raw denominator row stays in PSUM for fin_b's rec
            pacc = paccs[n]
            projn_u = work.tile([64, 512], MDT, tag="projn", name=f"pn{n}")
            nc.scalar.activation(projn_u[:], pacc[0:64, :],
                                 mybir.ActivationFunctionType.Identity)
            fin[n] = (projn_u,)

        def fin_b(n):
            # per-query 1/denom straight off PSUM (fast custom-DVE approx).
            # The custom op ignores input partition offsets, so run it over
            # all 65 partitions (same cost: DVE time = free size) and use
            # row 64. Then a tiny bf16 convert on Pool for the broadcast.
            pacc = paccs.pop(n)
            (projn_u,) = fin[n]
            rec = work.tile([96, 512], F32, tag="rec", name=f"rec{n}")
            nc.vector.reciprocal_approx_fast(out=rec[:], in_=pacc[:, :])
            recb = work.tile([1, 512], MDT, tag="recb", name=f"recb{n}")
            nc.scalar.activation(recb[:], rec[64:65, :],
                                 mybir.ActivationFunctionType.Identity)
            fin[n] = (projn_u, recb)

        def fin_c(n):
            # PE: broadcast 1/denom to [64,512] + out-projection
            projn_u, recb = fin[n]
            bc_ps = aux_ps.tile([64, 512], F32, tag="bcq", name=f"bc{n}")
            nc.tensor.matmul(bc_ps[:], ones_sb[0:1, 0:64], recb[:],
                             start=True, stop=True)
            fps = aux_ps.tile([64, 512], F32, tag="fpq", name=f"fps{n}")
            nc.tensor.matmul(fps[:], wo_sb[:], projn_u[:], start=True,
                             stop=True)
            bc_sb = work.tile([64, 512], F32, tag="bc", name=f"bcs{n}")
            nc.scalar.activation(bc_sb[:], bc_ps[:],
                                 mybir.ActivationFunctionType.Identity)
            fin[n] = (fps, bc_sb)

        def fin_d(n):
            # normalize on DVE, then bias + residual + store on Pool
            fps, bc_sb = fin.pop(n)
            qsl = slice(512 * n, 512 * n + 512)
            mn = work.tile([64, 512], F32, tag="mn", name=f"mn{n}")
            nc.vector.tensor_mul(mn[:], fps[:], bc_sb[:])
            # last tile's residual-add on DVE: it is the serial kernel tail
            eng = nc.vector if n == 3 else nc.gpsimd
            eng.tensor_add(out_sb[:, qsl], mn[:], x_sb[0:64, qsl])
            nc.sync.dma_start(out_d.ap()[:, qsl], out_sb[:, qsl])

        # ---- software-pipelined attention ----
        # tile 0 extras: qk slices 1-3 and v chunks produced just in time
        # (scores pair p needs kt slice p//4, attnV pair p-LAG needs v chunk
        # p-LAG). The earliest qkv psums ride the pacc-tag banks (free until
        # the first pacc allocation at p=LAG); the rest alternate bcq/fpq so
        # every tenant's copy has >= 2 pairs to drain before bank reuse.
        PACC_TAG = (pacc_ps, "pacc")
        T0_EXTRA = {0: [("qk", 1, (PACC_TAG, PACC_TAG))],
                    1: [("v", 0, PACC_TAG), ("v", 1, PACC_TAG)],
                    2: [("v", 2, None), ("v", 3, None)],
                    3: [("qk", 2, None)],
                    4: [("v", 4, None), ("v", 5, None)],
                    5: [("v", 6, None)], 6: [("v", 7, None)],
                    7: [("qk", 3, None)],
                    8: [("v", 8, None)], 9: [("v", 9, None)],
                    10: [("v", 10, None)], 11: [("v", 11, None)],
                    12: [("v", 12, None)], 13: [("v", 13, None)],
                    14: [("v", 14, None)], 15: [("v", 15, None)]}
        # tiles 1-3: previous tile's spill attnV pairs + finish steps (spread
        # out so each step's engine work has slack before its consumer)
        TN_EXTRA = {0: [("spill", 10)], 1: [("spill", 11)],
                    2: [("spill", 12)], 3: [("spill", 13)],
                    4: [("spill", 14)], 5: [("spill", 15)],
                    6: [("fina",)], 7: [("finb",)],
                    9: [("finc",)], 11: [("find",)]}
        AUX = ((aux_ps, "bcq"), (aux_ps, "fpq"))

        emit_qk_slice(0, AUX)
        for n in range(4):
            for p in range(16):
                ps = emit_scores(n, p)
                if p >= LAGS[n]:
                    emit_attnv(n, p - LAGS[n])
                if n == 0:
                    for item in T0_EXTRA.get(p, []):
                        if item[0] == "qk":
                            emit_qk_slice(item[1], item[2] or AUX)
                        else:
                            emit_v(item[1], item[2])
                else:
                    for item in TN_EXTRA.get(p, []):
                        if item[0] == "spill":
                            emit_attnv(n - 1, item[1])
                        elif item[0] == "fina":
                            fin_a(n - 1)
                        elif item[0] == "finb":
                            fin_b(n - 1)
                        elif item[0] == "finc":
                            fin_c(n - 1)
                        else:
                            fin_d(n - 1)
                emit_exp(n, p, ps)
        for p in range(16 - LAGS[3], 16):
            emit_attnv(3, p)
        fin_a(3)
        fin_b(3)
        fin_c(3)
        fin_d(3)

    nc.compile()
    return nc


def host_prep(x, gamma, beta, Wq, bq, Wk, bk, Wv, bv, Wo, bo):
    """Build the 8 per-core input dicts."""
    f32 = lambda a: np.ascontiguousarray(np.asarray(a, np.float32))
    x = f32(x)
    gamma, beta = f32(gamma), f32(beta)
    Wq, Wk, Wv, Wo = f32(Wq), f32(Wk), f32(Wv), f32(Wo)
    bq, bk, bv, bo = f32(bq), f32(bk), f32(bv), f32(bo)

    wq_dup = np.ascontiguousarray(np.concatenate([Wq, Wq], axis=1))
    z = np.zeros((64, 64), np.float32)
    wk_blk = np.ascontiguousarray(np.block([[Wk, z], [z, Wk]]))
    wv_blk = np.ascontiguousarray(np.block([[Wv, z], [z, Wv]]))
    comb = np.zeros((128, 128), np.float32)
    comb[:64, :64] = 1.0 / 64.0
    comb[64:, 64:] = 1.0 / 64.0
    bo_f = bv @ Wo + bo  # fold v bias through the out-projection
    mdt_np = mybir.dt.np(MDT)
    m = lambda a: np.ascontiguousarray(a).astype(mdt_np)
    shared = {
        "wq": m(wq_dup), "wk": m(wk_blk), "wv": m(wv_blk), "wo": m(Wo),
        "bq": np.ascontiguousarray(np.tile(bq, 2)[:, None]),
        "bo": np.ascontiguousarray(
            np.concatenate([bo_f, np.zeros(64, np.float32)])[:, None]),
        "gam": np.ascontiguousarray(np.tile(gamma, 2)[:, None]),
        "bet": np.ascontiguousarray(np.tile(beta, 2)[:, None]),
        "comb": comb,
    }
    in_maps = []
    for core in range(8):
        b, h = core // 2, core % 2
        xT = x[b].reshape(HW, C).T  # [64, 4096]
        halves = xT.reshape(C, 2, HALF)[:, [h, 1 - h], :]
        xp = np.ascontiguousarray(halves.transpose(1, 0, 2).reshape(128, HALF))
        in_maps.append({"xp": xp, **shared})
    return in_maps


def assemble(results, dtype):
    out = np.empty((B, HW, C), np.float32)
    for core in range(8):
        b, h = core // 2, core % 2
        out[b, HALF * h: HALF * h + HALF] = results[core]["out"].T
    return out.reshape(B, H, W, C).astype(dtype, copy=False)


_NC_CACHE = []


def kernel(x, gamma, beta, Wq, bq, Wk, bk, Wv, bv, Wo, bo):
    from concourse.bass_utils import run_bass_kernel_spmd

    if not _NC_CACHE:
        _NC_CACHE.append(build_nc())
    nc = _NC_CACHE[0]
    in_maps = host_prep(x, gamma, beta, Wq, bq, Wk, bk, Wv, bv, Wo, bo)
    res = run_bass_kernel_spmd(nc, in_maps, core_ids=list(range(8)))
    return assemble(res.results, np.asarray(x).dtype)


if __name__ == "__main__":
    rng = np.random.default_rng(0)
    inputs = {
        "x": rng.standard_normal((B, H, W, C)).astype(np.float32),
        "gamma": np.ones(C, np.float32), "beta": np.zeros(C, np.float32),
        "Wq": (rng.standard_normal((C, C)) / 8).astype(np.float32),
        "bq": np.zeros(C, np.float32),
        "Wk": (rng.standard_normal((C, C)) / 8).astype(np.float32),
        "bk": np.zeros(C, np.float32),
        "Wv": (rng.standard_normal((C, C)) / 8).astype(np.float32),
        "bv": np.zeros(C, np.float32),
        "Wo": (rng.standard_normal((C, C)) / 8).astype(np.float32),
        "bo": np.zeros(C, np.float32),
    }
    out = kernel(**inputs)
    print("kernel ran, out shape", out.shape, out.dtype)


# revision 55
# speedup vs baseline: 1.1514x; 1.1514x over previous
"""Trainium2 Bass kernel for nn_AttentionBlock (B=4, H=W=64, C=64, GroupNorm(8) +
full spatial self-attention), distributed over 8 NeuronCores.

Sharding: core i handles batch b=i//2 and query-half h=i%2 (2048 of the 4096
spatial positions). Each core computes the full GroupNorm and K/V for its
image (cheap) and attention only for its query half. No collectives.

v2 pipeline:
- exp split across THREE engines: ACT (table exp) for some score pairs,
  Pool (gpsimd) and DVE for the rest via a single-op int16 Schraudolph
  (i16 = s*23.083 + 16256.5 truncated, bitcast bf16 ~= e^(s/8), max rel err
  ~4%, final output err ~3e-3; denominator uses the same approximated
  weights so softmax normalization stays consistent).
- PE stream is gap-free: warmup matmuls ramp the clock during GroupNorm
  stats, then per tile scores-pair p / attnV pair p-3 alternate, with
  qk/v production and prev-tile finish matmuls slotted into the bubbles.
- biases: bq/bk folded into the q/k PSUM->SBUF copies (per-partition add);
  bv folded into bo on the host (bv @ Wo + bo).
- softmax denominators ride as a 65th ones-column of V; reciprocal via
  the fast custom-DVE op on [1,512] then broadcast by a bf16 PE matmul.
"""

import sys

sys.path.insert(0, "/opt/trn_rl_repo")

import numpy as np

import concourse.bacc as bacc
import concourse.tile as tile
from concourse import mybir

B, H, W, C = 4, 64, 64, 64
HW = H * W  # 4096
HALF = HW // 2  # 2048
EPS = 1e-5
SCALE = C ** -0.5

F32 = mybir.dt.float32
MDT = mybir.dt.bfloat16  # PE matmul operand dtype (scores/projections)
I16 = mybir.dt.int16
I8 = mybir.dt.int8
F8E5 = mybir.dt.float8e5  # attn weights (e5m2: range to 57344 covers e^9)
F8E4 = mybir.dt.float8e4  # v values (e4m3)

# Schraudolph exp in bf16-bit space: i16 = round(s * 2^7/ln2 * SCALE + 127*2^7)
SCH_SCALE = float((2.0 ** 7) / np.log(2.0) * SCALE)
SCH_BIAS = 16251.0  # 127*2^7 shifted -5.5 to center the one-sided
# mantissa-interpolation error (+0..6.7%) around zero
# same trick in e5m2-bit space: i8 = s * 2^2/ln2 * SCALE + 15*2^2
SCH8_SCALE = float(4.0 / np.log(2.0) * SCALE)
SCH8_BIAS = 60.25

NWARM = 24  # PE warmup matmuls (ramp p-state during GN stats)
LAGS = [6, 6, 6, 3]  # attnV trails scores by LAG pairs; short last tile
# so the post-loop drain is small

# engine per exp pair: A=ACT table exp, D=DVE int16-schraudolph. (Pool cannot
# read PSUM on TRN2, so it only gets SBUF->SBUF work: xn, recb, final out.)
# Tile tails lean A so DVE is clear for the next tile's start.
EMAP0 = ['D', 'A', 'D', 'A', 'D', 'A', 'D', 'A',
         'D', 'A', 'D', 'A', 'A', 'D', 'A', 'A']      # A9 D7
EMAPN = ['D', 'A', 'D', 'A', 'D', 'A', 'D', 'A',
         'D', 'D', 'A', 'D', 'D', 'A', 'A', 'A']      # A8 D8
EMAP3 = ['D', 'A'] * 8  # strict alternation: tile 3 runs LAG=3, so each
# exp must land within ~2 pairs of its scores
EMAPS = [EMAP0, EMAPN, EMAPN, EMAP3]


def build_nc():
    nc = bacc.Bacc("TRN2", debug=False, num_devices=8)

    # ---- DRAM I/O ----
    xp_d = nc.dram_tensor("xp", [128, HALF], F32, kind="ExternalInput")
    wq_d = nc.dram_tensor("wq", [64, 128], MDT, kind="ExternalInput")
    wk_d = nc.dram_tensor("wk", [128, 128], MDT, kind="ExternalInput")
    wv_d = nc.dram_tensor("wv", [128, 128], MDT, kind="ExternalInput")
    wo_d = nc.dram_tensor("wo", [64, 64], MDT, kind="ExternalInput")
    bq_d = nc.dram_tensor("bq", [128, 1], F32, kind="ExternalInput")
    bo_d = nc.dram_tensor("bo", [128, 1], F32, kind="ExternalInput")
    gam_d = nc.dram_tensor("gam", [128, 1], F32, kind="ExternalInput")
    bet_d = nc.dram_tensor("bet", [128, 1], F32, kind="ExternalInput")
    comb_d = nc.dram_tensor("comb", [128, 128], F32, kind="ExternalInput")
    out_d = nc.dram_tensor("out", [64, HALF], F32, kind="ExternalOutput")

    with tile.TileContext(nc) as tc, \
         tc.tile_pool(name="singles", bufs=1) as singles, \
         tc.tile_pool(name="stats", bufs=1) as stats, \
         tc.tile_pool(name="sc_ps", bufs=2, space="PSUM") as sc_ps, \
         tc.tile_pool(name="pacc_ps", bufs=2, space="PSUM") as pacc_ps, \
         tc.tile_pool(name="aux_ps", bufs=1, space="PSUM") as aux_ps, \
         tc.tile_pool(name="work", bufs=2) as work:

        # ---- input DMAs: everything on the sync hwdge queue (the issuing
        # engine is otherwise idle; putting DMAs on the ACT queue would
        # block the sqrt/exp table work behind descriptor writes) ----
        x_sb = singles.tile([128, HALF], F32)
        for r in range(4):
            eng = nc.sync if r % 2 == 0 else nc.scalar
            eng.dma_start(
                x_sb[:, 512 * r: 512 * r + 512],
                xp_d.ap()[:, 512 * r: 512 * r + 512],
            )
        gam_sb = singles.tile([128, 1], F32)
        nc.sync.dma_start(gam_sb[:], gam_d.ap())
        bet_sb = singles.tile([128, 1], F32)
        nc.sync.dma_start(bet_sb[:], bet_d.ap())
        comb_sb = singles.tile([128, 128], F32)
        nc.sync.dma_start(comb_sb[:], comb_d.ap())
        wk_sb = singles.tile([128, 128], MDT)
        nc.sync.dma_start(wk_sb[:], wk_d.ap())
        wq_sb = singles.tile([64, 128], MDT)
        nc.sync.dma_start(wq_sb[:], wq_d.ap())
        bq_sb = singles.tile([128, 1], F32)
        nc.sync.dma_start(bq_sb[:], bq_d.ap())
        wv_sb = singles.tile([128, 128], MDT)
        nc.sync.dma_start(wv_sb[:], wv_d.ap())
        wo_sb = singles.tile([64, 64], MDT)
        nc.sync.dma_start(wo_sb[:], wo_d.ap())
        bo_sb = singles.tile([128, 1], F32)
        nc.sync.dma_start(bo_sb[:], bo_d.ap())

        # ---- big SBUF tensors ----
        xn_r = singles.tile([128, HALF], MDT)
        q_dup = singles.tile([128, HALF], MDT)
        kt_sb = singles.tile([128, HALF], MDT)
        v_all = singles.tile([128, 65 * 32], MDT)
        attnexp = singles.tile([128, 1024 * 16], MDT)
        out_sb = singles.tile([64, HALF], F32)
        ones_sb = singles.tile([128, 512], MDT)

        # constants on Pool, first thing (warmup matmuls read ones_sb)
        nc.gpsimd.memset(ones_sb[:], 1.0)
        v4 = v_all[:].rearrange("p (h t e) -> p h t e", h=2, e=65)
        nc.gpsimd.memset(v4[:, :, :, 64:65], 1.0)

        # pre-warm the sqrt ACT table set (used by the GN rstd); the exp
        # table is loaded right after the single sqrt below
        scr = stats.tile([128, 1], F32)
        nc.vector.memset(scr[:], 1.0)
        nc.scalar.activation(scr[:], scr[:], mybir.ActivationFunctionType.Sqrt)

        # ---- PE warmup: ramp the activity monitor while DVE does GN stats
        # (rides the sc-tag banks, which are free until the first scores) ----
        for w in range(NWARM):
            wps = sc_ps.tile([128, 512], F32, tag="sc", name=f"warm{w}")
            nc.tensor.matmul(wps[:], ones_sb[:, 0:128], ones_sb[:, :],
                             start=True, stop=True)

        # ---- GroupNorm stats: bn per partition per 512-slice, then a
        # block-diagonal averaging matmul combines across channels ----
        st6 = stats.tile([128, 4, 6], F32)
        mv4 = stats.tile([128, 4, 2], F32)
        for r in range(4):
            nc.vector.bn_stats(st6[:, r, :], x_sb[:, 512 * r: 512 * r + 512])
            nc.vector.bn_aggr(mv4[:, r, :], st6[:, r, :])
        smat = stats.tile([128, 8], F32)  # cols 0-3 mean, 4-7 E[x^2]
        nc.vector.tensor_copy(smat[:, 0:4], mv4[:, :, 0])
        nc.vector.tensor_mul(smat[:, 4:8], mv4[:, :, 0], mv4[:, :, 0])
        nc.vector.tensor_add(smat[:, 4:8], smat[:, 4:8], mv4[:, :, 1])

        cps = pacc_ps.tile([128, 8], F32, tag="pacc")
        nc.tensor.matmul(cps[:], comb_sb[:], smat[:], start=True, stop=True)
        gstat = stats.tile([128, 8], F32)  # 0-3 mean_g, 4-7 E2_g
        nc.vector.tensor_copy(gstat[:], cps[:])

        # var = E2 - mean^2; EPS folds into the sqrt's activation bias.
        # rstd = 1/sqrt(var+EPS) via ACT sqrt + fast custom-DVE reciprocal
        # (the sanctioned accurate path; far fewer serial ops than a
        # bit-trick Newton chain)
        ve = stats.tile([128, 4], F32)
        nc.vector.tensor_mul(ve[:], gstat[:, 0:4], gstat[:, 0:4])
        nc.vector.tensor_sub(ve[:], gstat[:, 4:8], ve[:])
        eps_sb = stats.tile([128, 1], F32)
        nc.vector.memset(eps_sb[:], EPS)
        sve = stats.tile([128, 4], F32)
        nc.scalar.activation(sve[:], ve[:],
                             mybir.ActivationFunctionType.Sqrt,
                             bias=eps_sb[:])
        # switch ACT to the exp table now, during idle time
        nc.scalar.activation(scr[:], scr[:], mybir.ActivationFunctionType.Exp)
        rstd = stats.tile([128, 4], F32)
        nc.vector.reciprocal_approx_fast(out=rstd[:], in_=sve[:])

        gsc = stats.tile([128, 4], F32)
        nc.vector.tensor_scalar_mul(gsc[:], rstd[:], gam_sb[:])
        gbias = stats.tile([128, 4], F32)
        nc.vector.tensor_mul(gbias[:], gstat[:, 0:4], gsc[:])
        nc.vector.tensor_scalar(
            out=gbias[:], in0=gbias[:], scalar1=-1.0, scalar2=bet_sb[:],
            op0=mybir.AluOpType.mult, op1=mybir.AluOpType.add,
        )
        # the fp32 residual pass folds in bo (bo rides rows 0:63 of the bias;
        # rows 64:127 of x_sb are never read again after the qkv matmuls)
        gbias2 = stats.tile([128, 4], F32)
        nc.vector.tensor_scalar_add(gbias2[:], gbias[:], bo_sb[:])
        # xn = x * gsc + gbias: slice 0's bf16 copy runs on DVE (it gates
        # qk0 -> first scores); the rest and the fp32 residual pass run on
        # the slow-but-idle Pool. Per slice: bf16 read first, then the
        # in-place fp32 overwrite (Pool ops ordered; DVE xn0 emitted before
        # Pool's slice-0 overwrite so the framework serializes the WAR).
        nc.vector.tensor_scalar(
            out=xn_r[:, 0:512], in0=x_sb[:, 0:512],
            scalar1=gsc[:, 0:1], scalar2=gbias[:, 0:1],
            op0=mybir.AluOpType.mult, op1=mybir.AluOpType.add,
        )
        for r in range(4):
            sl = slice(512 * r, 512 * r + 512)
            if r > 0:
                nc.gpsimd.tensor_scalar(
                    out=xn_r[:, sl], in0=x_sb[:, sl],
                    scalar1=gsc[:, r: r + 1], scalar2=gbias[:, r: r + 1],
                    op0=mybir.AluOpType.mult, op1=mybir.AluOpType.add,
                )
            nc.gpsimd.tensor_scalar(
                out=x_sb[:, sl], in0=x_sb[:, sl],
                scalar1=gsc[:, r: r + 1], scalar2=gbias2[:, r: r + 1],
                op0=mybir.AluOpType.mult, op1=mybir.AluOpType.add,
            )

        # ---- emission helpers ----
        def emit_qk_slice(t, pool_tags):
            # k^T packed by half (lhsT = blockdiag(Wk, Wk)); q^T duplicated on
            # both partition halves (lhsT = [Wq | Wq]). bk is dropped exactly
            # (a per-query score constant cancels in softmax); bq folds into
            # the q copy as a per-partition bias on ACT.
            sl = slice(512 * t, 512 * t + 512)
            pool_k, tag_k = pool_tags[0]
            pool_q, tag_q = pool_tags[1]
            ps2 = pool_k.tile([128, 512], F32, tag=tag_k, name=f"kps{t}")
            nc.tensor.matmul(ps2[:], wk_sb[:], xn_r[:, sl], start=True,
                             stop=True)
            nc.scalar.activation(kt_sb[:, sl], ps2[:],
                                 mybir.ActivationFunctionType.Identity)
            ps = pool_q.tile([128, 512], F32, tag=tag_q, name=f"qps{t}")
            nc.tensor.matmul(ps[:], wq_sb[:], xn_r[0:64, sl], start=True,
                             stop=True)
            nc.scalar.activation(
                q_dup[:, sl], ps[:], mybir.ActivationFunctionType.Identity,
                bias=bq_sb[:],
            )

        def emit_v(u, pool_tag=None):
            # v position-major, two 128-position chunks (halves) per matmul;
            # single copy with a dual-chunk strided output AP. Even u on the
            # bcq psum tag + DVE copy, odd u on fpq + ACT.
            sl = slice(128 * u, 128 * u + 128)
            pool, tag = pool_tag or (aux_ps, "bcq" if u % 2 == 0 else "fpq")
            ps = pool.tile([128, 128], F32, tag=tag, name=f"vps{u}")
            nc.tensor.matmul(ps[:], xn_r[:, sl], wv_sb[:], start=True,
                             stop=True)
            psr = ps[:].rearrange("p (h e) -> p h e", h=2)
            nc.vector.tensor_copy(v4[:, :, u, 0:64], psr[:, :, :])

        def emit_scores(n, p):
            # pair p: kv chunks p (half0, PE rows 0-63) and p+16 (half1, rows
            # 64-127) run concurrently; one [128,1024] 2-bank psum tile
            qsl = slice(512 * n, 512 * n + 512)
            ksl = slice(128 * p, 128 * p + 128)
            ps = sc_ps.tile([128, 1024], F32, tag="sc", name=f"sc{n}_{p}")
            nc.tensor.matmul(ps[:, 0:512], kt_sb[0:64, ksl],
                             q_dup[0:64, qsl], start=True, stop=True)
            nc.tensor.matmul(ps[:, 512:1024], kt_sb[64:128, ksl],
                             q_dup[64:128, qsl], start=True, stop=True)
            return ps

        def emit_exp(n, p, ps):
            # attnexp layout pair-major: chunk p at 1024p, chunk p+16 at
            # 1024p+512 — both written by this single instruction
            dst = attnexp[:, 1024 * p: 1024 * p + 1024]
            e = EMAPS[n][p]
            if e == 'A':
                nc.scalar.activation(dst, ps[:],
                                     mybir.ActivationFunctionType.Exp,
                                     scale=SCALE)
            else:
                nc.vector.tensor_scalar(
                    out=dst.bitcast(I16), in0=ps[:],
                    scalar1=SCH_SCALE, scalar2=SCH_BIAS,
                    op0=mybir.AluOpType.mult, op1=mybir.AluOpType.add,
                )

        paccs = {}

        def emit_attnv(n, p):
            # kv chunk pair (p, p+16) — consumes exp pair p. One fp8
            # DoubleRow matmul per pair: contraction 2x128 kv, 0.5 cyc/row.
            if n not in paccs:
                paccs[n] = pacc_ps.tile([65, 512], F32, tag="pacc",
                                        name=f"pacc{n}")
            pacc = paccs[n]
            for t in (p, p + 16):
                off = 1024 * p + (512 if t >= 16 else 0)
                nc.tensor.matmul(
                    pacc[:], v_all[:, 65 * t: 65 * t + 65],
                    attnexp[:, off: off + 512],
                    start=(t == 0), stop=(t == 31),
                )

        # finish chain for tile n, split into steps scheduled across pairs of
        # tile n+1 so the PE stream stays dense
        fin = {}

        def fin_a(n):
            # free the PSUM accumulator ASAP: unnormalized proj rows (bf16)
            # on ACT; the raw denominator row stays in PSUM for fin_b's rec
            pacc = paccs[n]
            projn_u = work.tile([64, 512], MDT, tag="projn", name=f"pn{n}")
            nc.scalar.activation(projn_u[:], pacc[0:64, :],
                                 mybir.ActivationFunctionType.Identity)
            fin[n] = (projn_u,)

        def fin_b(n):
            # per-query 1/denom straight off PSUM (fast custom-DVE approx).
            # The custom op ignores input partition offsets, so run it over
            # all 65 partitions (same cost: DVE time = free size) and use
            # row 64. Then a tiny bf16 convert on Pool for the broadcast.
            pacc = paccs.pop(n)
            (projn_u,) = fin[n]
            rec = work.tile([65, 512], F32, tag="rec", name=f"rec{n}")
            nc.vector.reciprocal_approx_fast(out=rec[:], in_=pacc[:, :])
            recb = work.tile([1, 512], MDT, tag="recb", name=f"recb{n}")
            nc.scalar.activation(recb[:], rec[64:65, :],
                                 mybir.ActivationFunctionType.Identity)
            fin[n] = (projn_u, recb)

        def fin_c(n):
            # PE: broadcast 1/denom to [64,512] + out-projection
            projn_u, recb = fin[n]
            bc_ps = aux_ps.tile([64, 512], F32, tag="bcq", name=f"bc{n}")
            nc.tensor.matmul(bc_ps[:], ones_sb[0:1, 0:64], recb[:],
                             start=True, stop=True)
            fps = aux_ps.tile([64, 512], F32, tag="fpq", name=f"fps{n}")
            nc.tensor.matmul(fps[:], wo_sb[:], projn_u[:], start=True,
                             stop=True)
            bc_sb = work.tile([64, 512], F32, tag="bc", name=f"bcs{n}")
            nc.scalar.activation(bc_sb[:], bc_ps[:],
                                 mybir.ActivationFunctionType.Identity)
            fin[n] = (fps, bc_sb)

        def fin_d(n):
            # normalize on DVE, then bias + residual + store on Pool
            fps, bc_sb = fin.pop(n)
            qsl = slice(512 * n, 512 * n + 512)
            mn = work.tile([64, 512], F32, tag="mn", name=f"mn{n}")
            nc.vector.tensor_mul(mn[:], fps[:], bc_sb[:])
            # last tile's residual-add on DVE: it is the serial kernel tail
            eng = nc.vector if n == 3 else nc.gpsimd
            eng.tensor_add(out_sb[:, qsl], mn[:], x_sb[0:64, qsl])
            nc.sync.dma_start(out_d.ap()[:, qsl], out_sb[:, qsl])

        # ---- software-pipelined attention ----
        # tile 0 extras: qk slices 1-3 and v chunks produced just in time
        # (scores pair p needs kt slice p//4, attnV pair p-LAG needs v chunk
        # p-LAG). The earliest qkv psums ride the pacc-tag banks (free until
        # the first pacc allocation at p=LAG); the rest alternate bcq/fpq so
        # every tenant's copy has >= 2 pairs to drain before bank reuse.
        PACC_TAG = (pacc_ps, "pacc")
        T0_EXTRA = {0: [("qk", 1, (PACC_TAG, PACC_TAG))],
                    1: [("v", 0, PACC_TAG), ("v", 1, PACC_TAG)],
                    2: [("v", 2, None), ("v", 3, None)],
                    3: [("qk", 2, None)],
                    4: [("v", 4, None), ("v", 5, None)],
                    5: [("v", 6, None)], 6: [("v", 7, None)],
                    7: [("qk", 3, None)],
                    8: [("v", 8, None)], 9: [("v", 9, None)],
                    10: [("v", 10, None)], 11: [("v", 11, None)],
                    12: [("v", 12, None)], 13: [("v", 13, None)],
                    14: [("v", 14, None)], 15: [("v", 15, None)]}
        # tiles 1-3: previous tile's spill attnV pairs + finish steps (spread
        # out so each step's engine work has slack before its consumer)
        TN_EXTRA = {0: [("spill", 10)], 1: [("spill", 11)],
                    2: [("spill", 12)], 3: [("spill", 13)],
                    4: [("spill", 14)], 5: [("spill", 15)],
                    6: [("fina",)], 7: [("finb",)],
                    9: [("finc",)], 11: [("find",)]}
        AUX = ((aux_ps, "bcq"), (aux_ps, "fpq"))

        emit_qk_slice(0, AUX)
        for n in range(4):
            for p in range(16):
                ps = emit_scores(n, p)
                if p >= LAGS[n]:
                    emit_attnv(n, p - LAGS[n])
                if n == 0:
                    for item in T0_EXTRA.get(p, []):
                        if item[0] == "qk":
                            emit_qk_slice(item[1], item[2] or AUX)
                        else:
                            emit_v(item[1], item[2])
                else:
                    for item in TN_EXTRA.get(p, []):
                        if item[0] == "spill":
                            emit_attnv(n - 1, item[1])
                        elif item[0] == "fina":
                            fin_a(n - 1)
                        elif item[0] == "finb":
                            fin_b(n - 1)
                        elif item[0] == "finc":
                            fin_c(n - 1)
                        else:
                            fin_d(n - 1)
                emit_exp(n, p, ps)
        for p in range(16 - LAGS[3], 16):
            emit_attnv(3, p)
        fin_a(3)
        fin_b(3)
        fin_c(3)
        fin_d(3)

    nc.compile()
    return nc


def host_prep(x, gamma, beta, Wq, bq, Wk, bk, Wv, bv, Wo, bo):
    """Build the 8 per-core input dicts."""
    f32 = lambda a: np.ascontiguousarray(np.asarray(a, np.float32))
    x = f32(x)
    gamma, beta = f32(gamma), f32(beta)
    Wq, Wk, Wv, Wo = f32(Wq), f32(Wk), f32(Wv), f32(Wo)
    bq, bk, bv, bo = f32(bq), f32(bk), f32(bv), f32(bo)

    wq_dup = np.ascontiguousarray(np.concatenate([Wq, Wq], axis=1))
    z = np.zeros((64, 64), np.float32)
    wk_blk = np.ascontiguousarray(np.block([[Wk, z], [z, Wk]]))
    wv_blk = np.ascontiguousarray(np.block([[Wv, z], [z, Wv]]))
    comb = np.zeros((128, 128), np.float32)
    comb[:64, :64] = 1.0 / 64.0
    comb[64:, 64:] = 1.0 / 64.0
    bo_f = bv @ Wo + bo  # fold v bias through the out-projection
    mdt_np = mybir.dt.np(MDT)
    m = lambda a: np.ascontiguousarray(a).astype(mdt_np)
    shared = {
        "wq": m(wq_dup), "wk": m(wk_blk), "wv": m(wv_blk), "wo": m(Wo),
        "bq": np.ascontiguousarray(np.tile(bq, 2)[:, None]),
        "bo": np.ascontiguousarray(
            np.concatenate([bo_f, np.zeros(64, np.float32)])[:, None]),
        "gam": np.ascontiguousarray(np.tile(gamma, 2)[:, None]),
        "bet": np.ascontiguousarray(np.tile(beta, 2)[:, None]),
        "comb": comb,
    }
    in_maps = []
    for core in range(8):
        b, h = core // 2, core % 2
        xT = x[b].reshape(HW, C).T  # [64, 4096]
        halves = xT.reshape(C, 2, HALF)[:, [h, 1 - h], :]
        xp = np.ascontiguousarray(halves.transpose(1, 0, 2).reshape(128, HALF))
        in_maps.append({"xp": xp, **shared})
    return in_maps


def assemble(results, dtype):
    out = np.empty((B, HW, C), np.float32)
    for core in range(8):
        b, h = core // 2, core % 2
        out[b, HALF * h: HALF * h + HALF] = results[core]["out"].T
    return out.reshape(B, H, W, C).astype(dtype, copy=False)


_NC_CACHE = []


def kernel(x, gamma, beta, Wq, bq, Wk, bk, Wv, bv, Wo, bo):
    from concourse.bass_utils import run_bass_kernel_spmd

    if not _NC_CACHE:
        _NC_CACHE.append(build_nc())
    nc = _NC_CACHE[0]
    in_maps = host_prep(x, gamma, beta, Wq, bq, Wk, bk, Wv, bv, Wo, bo)
    res = run_bass_kernel_spmd(nc, in_maps, core_ids=list(range(8)))
    return assemble(res.results, np.asarray(x).dtype)


if __name__ == "__main__":
    rng = np.random.default_rng(0)
    inputs = {
        "x": rng.standard_normal((B, H, W, C)).astype(np.float32),
        "gamma": np.ones(C, np.float32), "beta": np.zeros(C, np.float32),
        "Wq": (rng.standard_normal((C, C)) / 8).astype(np.float32),
        "bq": np.zeros(C, np.float32),
        "Wk": (rng.standard_normal((C, C)) / 8).astype(np.float32),
        "bk": np.zeros(C, np.float32),
        "Wv": (rng.standard_normal((C, C)) / 8).astype(np.float32),
        "bv": np.zeros(C, np.float32),
        "Wo": (rng.standard_normal((C, C)) / 8).astype(np.float32),
        "bo": np.zeros(C, np.float32),
    }
    out = kernel(**inputs)
    print("kernel ran, out shape", out.shape, out.dtype)


# revision 56
# speedup vs baseline: 1.1591x; 1.0067x over previous
"""Trainium2 Bass kernel for nn_AttentionBlock (B=4, H=W=64, C=64, GroupNorm(8) +
full spatial self-attention), distributed over 8 NeuronCores.

Sharding: core i handles batch b=i//2 and query-half h=i%2 (2048 of the 4096
spatial positions). Each core computes the full GroupNorm and K/V for its
image (cheap) and attention only for its query half. No collectives.

v2 pipeline:
- exp split across THREE engines: ACT (table exp) for some score pairs,
  Pool (gpsimd) and DVE for the rest via a single-op int16 Schraudolph
  (i16 = s*23.083 + 16256.5 truncated, bitcast bf16 ~= e^(s/8), max rel err
  ~4%, final output err ~3e-3; denominator uses the same approximated
  weights so softmax normalization stays consistent).
- PE stream is gap-free: warmup matmuls ramp the clock during GroupNorm
  stats, then per tile scores-pair p / attnV pair p-3 alternate, with
  qk/v production and prev-tile finish matmuls slotted into the bubbles.
- biases: bq/bk folded into the q/k PSUM->SBUF copies (per-partition add);
  bv folded into bo on the host (bv @ Wo + bo).
- softmax denominators ride as a 65th ones-column of V; reciprocal via
  the fast custom-DVE op on [1,512] then broadcast by a bf16 PE matmul.
"""

import sys

sys.path.insert(0, "/opt/trn_rl_repo")

import numpy as np

import concourse.bacc as bacc
import concourse.tile as tile
from concourse import mybir

B, H, W, C = 4, 64, 64, 64
HW = H * W  # 4096
HALF = HW // 2  # 2048
EPS = 1e-5
SCALE = C ** -0.5

F32 = mybir.dt.float32
MDT = mybir.dt.bfloat16  # PE matmul operand dtype (scores/projections)
I16 = mybir.dt.int16
I8 = mybir.dt.int8
F8E5 = mybir.dt.float8e5  # attn weights (e5m2: range to 57344 covers e^9)
F8E4 = mybir.dt.float8e4  # v values (e4m3)

# Schraudolph exp in bf16-bit space: i16 = round(s * 2^7/ln2 * SCALE + 127*2^7)
SCH_SCALE = float((2.0 ** 7) / np.log(2.0) * SCALE)
SCH_BIAS = 16251.0  # 127*2^7 shifted -5.5 to center the one-sided
# mantissa-interpolation error (+0..6.7%) around zero
# same trick in e5m2-bit space: i8 = s * 2^2/ln2 * SCALE + 15*2^2
SCH8_SCALE = float(4.0 / np.log(2.0) * SCALE)
SCH8_BIAS = 60.25

NWARM = 24  # PE warmup matmuls (ramp p-state during GN stats)
LAGS = [6, 6, 6, 3]  # attnV trails scores by LAG pairs; short last tile
# so the post-loop drain is small

# engine per exp pair: A=ACT table exp, D=DVE int16-schraudolph. (Pool cannot
# read PSUM on TRN2, so it only gets SBUF->SBUF work: xn, recb, final out.)
# Tile tails lean A so DVE is clear for the next tile's start.
EMAP0 = ['D', 'A', 'D', 'A', 'D', 'A', 'D', 'A',
         'D', 'A', 'D', 'A', 'A', 'D', 'A', 'A']      # A9 D7
EMAPN = ['D', 'A', 'D', 'A', 'D', 'A', 'D', 'A',
         'D', 'D', 'A', 'D', 'D', 'A', 'A', 'A']      # A8 D8
EMAP3 = ['D', 'A'] * 8  # strict alternation: tile 3 runs LAG=3, so each
# exp must land within ~2 pairs of its scores
EMAPS = [EMAP0, EMAPN, EMAPN, EMAP3]


def build_nc():
    nc = bacc.Bacc("TRN2", debug=False, num_devices=8)

    # ---- DRAM I/O ----
    xp_d = nc.dram_tensor("xp", [128, HALF], F32, kind="ExternalInput")
    wq_d = nc.dram_tensor("wq", [64, 128], MDT, kind="ExternalInput")
    wk_d = nc.dram_tensor("wk", [128, 128], MDT, kind="ExternalInput")
    wv_d = nc.dram_tensor("wv", [128, 128], MDT, kind="ExternalInput")
    wo_d = nc.dram_tensor("wo", [64, 64], MDT, kind="ExternalInput")
    bq_d = nc.dram_tensor("bq", [128, 1], F32, kind="ExternalInput")
    bo_d = nc.dram_tensor("bo", [128, 1], F32, kind="ExternalInput")
    gam_d = nc.dram_tensor("gam", [128, 1], F32, kind="ExternalInput")
    bet_d = nc.dram_tensor("bet", [128, 1], F32, kind="ExternalInput")
    comb_d = nc.dram_tensor("comb", [128, 128], F32, kind="ExternalInput")
    out_d = nc.dram_tensor("out", [64, HALF], F32, kind="ExternalOutput")

    with tile.TileContext(nc) as tc, \
         tc.tile_pool(name="singles", bufs=1) as singles, \
         tc.tile_pool(name="stats", bufs=1) as stats, \
         tc.tile_pool(name="sc_ps", bufs=2, space="PSUM") as sc_ps, \
         tc.tile_pool(name="pacc_ps", bufs=2, space="PSUM") as pacc_ps, \
         tc.tile_pool(name="aux_ps", bufs=1, space="PSUM") as aux_ps, \
         tc.tile_pool(name="work", bufs=2) as work:

        # ---- input DMAs: everything on the sync hwdge queue (the issuing
        # engine is otherwise idle; putting DMAs on the ACT queue would
        # block the sqrt/exp table work behind descriptor writes) ----
        x_sb = singles.tile([128, HALF], F32)
        for r in range(4):
            nc.sync.dma_start(
                x_sb[:, 512 * r: 512 * r + 512],
                xp_d.ap()[:, 512 * r: 512 * r + 512],
            )
        gam_sb = singles.tile([128, 1], F32)
        nc.sync.dma_start(gam_sb[:], gam_d.ap())
        bet_sb = singles.tile([128, 1], F32)
        nc.sync.dma_start(bet_sb[:], bet_d.ap())
        comb_sb = singles.tile([128, 128], F32)
        nc.sync.dma_start(comb_sb[:], comb_d.ap())
        wk_sb = singles.tile([128, 128], MDT)
        nc.sync.dma_start(wk_sb[:], wk_d.ap())
        wq_sb = singles.tile([64, 128], MDT)
        nc.sync.dma_start(wq_sb[:], wq_d.ap())
        bq_sb = singles.tile([128, 1], F32)
        nc.sync.dma_start(bq_sb[:], bq_d.ap())
        wv_sb = singles.tile([128, 128], MDT)
        nc.sync.dma_start(wv_sb[:], wv_d.ap())
        wo_sb = singles.tile([64, 64], MDT)
        nc.sync.dma_start(wo_sb[:], wo_d.ap())
        bo_sb = singles.tile([128, 1], F32)
        nc.sync.dma_start(bo_sb[:], bo_d.ap())

        # ---- big SBUF tensors ----
        xn_r = singles.tile([128, HALF], MDT)
        q_dup = singles.tile([128, HALF], MDT)
        kt_sb = singles.tile([128, HALF], MDT)
        v_all = singles.tile([128, 65 * 32], MDT)
        attnexp = singles.tile([128, 1024 * 16], MDT)
        out_sb = singles.tile([64, HALF], F32)
        ones_sb = singles.tile([128, 512], MDT)

        # constants on Pool, first thing (warmup matmuls read ones_sb)
        nc.gpsimd.memset(ones_sb[:], 1.0)
        v4 = v_all[:].rearrange("p (h t e) -> p h t e", h=2, e=65)
        nc.gpsimd.memset(v4[:, :, :, 64:65], 1.0)

        # pre-warm the sqrt ACT table set (used by the GN rstd); the exp
        # table is loaded right after the single sqrt below
        scr = stats.tile([128, 1], F32)
        nc.vector.memset(scr[:], 1.0)
        nc.scalar.activation(scr[:], scr[:], mybir.ActivationFunctionType.Sqrt)

        # ---- PE warmup: ramp the activity monitor while DVE does GN stats
        # (rides the sc-tag banks, which are free until the first scores) ----
        for w in range(NWARM):
            wps = sc_ps.tile([128, 512], F32, tag="sc", name=f"warm{w}")
            nc.tensor.matmul(wps[:], ones_sb[:, 0:128], ones_sb[:, :],
                             start=True, stop=True)

        # ---- GroupNorm stats: bn per partition per 512-slice, then a
        # block-diagonal averaging matmul combines across channels ----
        st6 = stats.tile([128, 4, 6], F32)
        mv4 = stats.tile([128, 4, 2], F32)
        for r in range(4):
            nc.vector.bn_stats(st6[:, r, :], x_sb[:, 512 * r: 512 * r + 512])
            nc.vector.bn_aggr(mv4[:, r, :], st6[:, r, :])
        smat = stats.tile([128, 8], F32)  # cols 0-3 mean, 4-7 E[x^2]
        nc.vector.tensor_copy(smat[:, 0:4], mv4[:, :, 0])
        nc.vector.tensor_mul(smat[:, 4:8], mv4[:, :, 0], mv4[:, :, 0])
        nc.vector.tensor_add(smat[:, 4:8], smat[:, 4:8], mv4[:, :, 1])

        cps = pacc_ps.tile([128, 8], F32, tag="pacc")
        nc.tensor.matmul(cps[:], comb_sb[:], smat[:], start=True, stop=True)
        gstat = stats.tile([128, 8], F32)  # 0-3 mean_g, 4-7 E2_g
        nc.vector.tensor_copy(gstat[:], cps[:])

        # var = E2 - mean^2; EPS folds into the sqrt's activation bias.
        # rstd = 1/sqrt(var+EPS) via ACT sqrt + fast custom-DVE reciprocal
        # (the sanctioned accurate path; far fewer serial ops than a
        # bit-trick Newton chain)
        ve = stats.tile([128, 4], F32)
        nc.vector.tensor_mul(ve[:], gstat[:, 0:4], gstat[:, 0:4])
        nc.vector.tensor_sub(ve[:], gstat[:, 4:8], ve[:])
        eps_sb = stats.tile([128, 1], F32)
        nc.vector.memset(eps_sb[:], EPS)
        sve = stats.tile([128, 4], F32)
        nc.scalar.activation(sve[:], ve[:],
                             mybir.ActivationFunctionType.Sqrt,
                             bias=eps_sb[:])
        # switch ACT to the exp table now, during idle time
        nc.scalar.activation(scr[:], scr[:], mybir.ActivationFunctionType.Exp)
        rstd = stats.tile([128, 4], F32)
        nc.vector.reciprocal_approx_fast(out=rstd[:], in_=sve[:])

        gsc = stats.tile([128, 4], F32)
        nc.vector.tensor_scalar_mul(gsc[:], rstd[:], gam_sb[:])
        gbias = stats.tile([128, 4], F32)
        nc.vector.tensor_mul(gbias[:], gstat[:, 0:4], gsc[:])
        nc.vector.tensor_scalar(
            out=gbias[:], in0=gbias[:], scalar1=-1.0, scalar2=bet_sb[:],
            op0=mybir.AluOpType.mult, op1=mybir.AluOpType.add,
        )
        # the fp32 residual pass folds in bo (bo rides rows 0:63 of the bias;
        # rows 64:127 of x_sb are never read again after the qkv matmuls)
        gbias2 = stats.tile([128, 4], F32)
        nc.vector.tensor_scalar_add(gbias2[:], gbias[:], bo_sb[:])
        # xn = x * gsc + gbias: slice 0's bf16 copy runs on DVE (it gates
        # qk0 -> first scores); the rest and the fp32 residual pass run on
        # the slow-but-idle Pool. Per slice: bf16 read first, then the
        # in-place fp32 overwrite (Pool ops ordered; DVE xn0 emitted before
        # Pool's slice-0 overwrite so the framework serializes the WAR).
        nc.vector.tensor_scalar(
            out=xn_r[:, 0:512], in0=x_sb[:, 0:512],
            scalar1=gsc[:, 0:1], scalar2=gbias[:, 0:1],
            op0=mybir.AluOpType.mult, op1=mybir.AluOpType.add,
        )
        for r in range(4):
            sl = slice(512 * r, 512 * r + 512)
            if r > 0:
                nc.gpsimd.tensor_scalar(
                    out=xn_r[:, sl], in0=x_sb[:, sl],
                    scalar1=gsc[:, r: r + 1], scalar2=gbias[:, r: r + 1],
                    op0=mybir.AluOpType.mult, op1=mybir.AluOpType.add,
                )
            nc.gpsimd.tensor_scalar(
                out=x_sb[:, sl], in0=x_sb[:, sl],
                scalar1=gsc[:, r: r + 1], scalar2=gbias2[:, r: r + 1],
                op0=mybir.AluOpType.mult, op1=mybir.AluOpType.add,
            )

        # ---- emission helpers ----
        def emit_qk_slice(t, pool_tags):
            # k^T packed by half (lhsT = blockdiag(Wk, Wk)); q^T duplicated on
            # both partition halves (lhsT = [Wq | Wq]). bk is dropped exactly
            # (a per-query score constant cancels in softmax); bq folds into
            # the q copy as a per-partition bias on ACT.
            sl = slice(512 * t, 512 * t + 512)
            pool_k, tag_k = pool_tags[0]
            pool_q, tag_q = pool_tags[1]
            ps2 = pool_k.tile([128, 512], F32, tag=tag_k, name=f"kps{t}")
            nc.tensor.matmul(ps2[:], wk_sb[:], xn_r[:, sl], start=True,
                             stop=True)
            nc.scalar.activation(kt_sb[:, sl], ps2[:],
                                 mybir.ActivationFunctionType.Identity)
            ps = pool_q.tile([128, 512], F32, tag=tag_q, name=f"qps{t}")
            nc.tensor.matmul(ps[:], wq_sb[:], xn_r[0:64, sl], start=True,
                             stop=True)
            nc.scalar.activation(
                q_dup[:, sl], ps[:], mybir.ActivationFunctionType.Identity,
                bias=bq_sb[:],
            )

        def emit_v(u, pool_tag=None):
            # v position-major, two 128-position chunks (halves) per matmul;
            # single copy with a dual-chunk strided output AP. Even u on the
            # bcq psum tag + DVE copy, odd u on fpq + ACT.
            sl = slice(128 * u, 128 * u + 128)
            pool, tag = pool_tag or (aux_ps, "bcq" if u % 2 == 0 else "fpq")
            ps = pool.tile([128, 128], F32, tag=tag, name=f"vps{u}")
            nc.tensor.matmul(ps[:], xn_r[:, sl], wv_sb[:], start=True,
                             stop=True)
            psr = ps[:].rearrange("p (h e) -> p h e", h=2)
            nc.vector.tensor_copy(v4[:, :, u, 0:64], psr[:, :, :])

        def emit_scores(n, p):
            # pair p: kv chunks p (half0, PE rows 0-63) and p+16 (half1, rows
            # 64-127) run concurrently; one [128,1024] 2-bank psum tile
            qsl = slice(512 * n, 512 * n + 512)
            ksl = slice(128 * p, 128 * p + 128)
            ps = sc_ps.tile([128, 1024], F32, tag="sc", name=f"sc{n}_{p}")
            nc.tensor.matmul(ps[:, 0:512], kt_sb[0:64, ksl],
                             q_dup[0:64, qsl], start=True, stop=True)
            nc.tensor.matmul(ps[:, 512:1024], kt_sb[64:128, ksl],
                             q_dup[64:128, qsl], start=True, stop=True)
            return ps

        def emit_exp(n, p, ps):
            # attnexp layout pair-major: chunk p at 1024p, chunk p+16 at
            # 1024p+512 — both written by this single instruction
            dst = attnexp[:, 1024 * p: 1024 * p + 1024]
            e = EMAPS[n][p]
            if e == 'A':
                nc.scalar.activation(dst, ps[:],
                                     mybir.ActivationFunctionType.Exp,
                                     scale=SCALE)
            else:
                nc.vector.tensor_scalar(
                    out=dst.bitcast(I16), in0=ps[:],
                    scalar1=SCH_SCALE, scalar2=SCH_BIAS,
                    op0=mybir.AluOpType.mult, op1=mybir.AluOpType.add,
                )

        paccs = {}

        def emit_attnv(n, p):
            # kv chunk pair (p, p+16) — consumes exp pair p. One fp8
            # DoubleRow matmul per pair: contraction 2x128 kv, 0.5 cyc/row.
            if n not in paccs:
                paccs[n] = pacc_ps.tile([65, 512], F32, tag="pacc",
                                        name=f"pacc{n}")
            pacc = paccs[n]
            for t in (p, p + 16):
                off = 1024 * p + (512 if t >= 16 else 0)
                nc.tensor.matmul(
                    pacc[:], v_all[:, 65 * t: 65 * t + 65],
                    attnexp[:, off: off + 512],
                    start=(t == 0), stop=(t == 31),
                )

        # finish chain for tile n, split into steps scheduled across pairs of
        # tile n+1 so the PE stream stays dense
        fin = {}

        def fin_a(n):
            # free the PSUM accumulator ASAP: unnormalized proj rows (bf16)
            # on ACT; the raw denominator row stays in PSUM for fin_b's rec
            pacc = paccs[n]
            projn_u = work.tile([64, 512], MDT, tag="projn", name=f"pn{n}")
            nc.scalar.activation(projn_u[:], pacc[0:64, :],
                                 mybir.ActivationFunctionType.Identity)
            fin[n] = (projn_u,)

        def fin_b(n):
            # per-query 1/denom straight off PSUM (fast custom-DVE approx).
            # The custom op ignores input partition offsets, so run it over
            # all 65 partitions (same cost: DVE time = free size) and use
            # row 64. Then a tiny bf16 convert on Pool for the broadcast.
            pacc = paccs.pop(n)
            (projn_u,) = fin[n]
            rec = work.tile([65, 512], F32, tag="rec", name=f"rec{n}")
            nc.vector.reciprocal_approx_fast(out=rec[:], in_=pacc[:, :])
            recb = work.tile([1, 512], MDT, tag="recb", name=f"recb{n}")
            nc.scalar.activation(recb[:], rec[64:65, :],
                                 mybir.ActivationFunctionType.Identity)
            fin[n] = (projn_u, recb)

        def fin_c(n):
            # PE: broadcast 1/denom to [64,512] + out-projection
            projn_u, recb = fin[n]
            bc_ps = aux_ps.tile([64, 512], F32, tag="bcq", name=f"bc{n}")
            nc.tensor.matmul(bc_ps[:], ones_sb[0:1, 0:64], recb[:],
                             start=True, stop=True)
            fps = aux_ps.tile([64, 512], F32, tag="fpq", name=f"fps{n}")
            nc.tensor.matmul(fps[:], wo_sb[:], projn_u[:], start=True,
                             stop=True)
            bc_sb = work.tile([64, 512], F32, tag="bc", name=f"bcs{n}")
            nc.scalar.activation(bc_sb[:], bc_ps[:],
                                 mybir.ActivationFunctionType.Identity)
            fin[n] = (fps, bc_sb)

        def fin_d(n):
            # normalize on DVE, then bias + residual + store on Pool
            fps, bc_sb = fin.pop(n)
            qsl = slice(512 * n, 512 * n + 512)
            mn = work.tile([64, 512], F32, tag="mn", name=f"mn{n}")
            nc.vector.tensor_mul(mn[:], fps[:], bc_sb[:])
            # last tile's residual-add on DVE: it is the serial kernel tail
            eng = nc.vector if n == 3 else nc.gpsimd
            eng.tensor_add(out_sb[:, qsl], mn[:], x_sb[0:64, qsl])
            nc.sync.dma_start(out_d.ap()[:, qsl], out_sb[:, qsl])

        # ---- software-pipelined attention ----
        # tile 0 extras: qk slices 1-3 and v chunks produced just in time
        # (scores pair p needs kt slice p//4, attnV pair p-LAG needs v chunk
        # p-LAG). The earliest qkv psums ride the pacc-tag banks (free until
        # the first pacc allocation at p=LAG); the rest alternate bcq/fpq so
        # every tenant's copy has >= 2 pairs to drain before bank reuse.
        PACC_TAG = (pacc_ps, "pacc")
        T0_EXTRA = {0: [("qk", 1, (PACC_TAG, PACC_TAG))],
                    1: [("v", 0, PACC_TAG), ("v", 1, PACC_TAG)],
                    2: [("v", 2, None), ("v", 3, None)],
                    3: [("qk", 2, None)],
                    4: [("v", 4, None), ("v", 5, None)],
                    5: [("v", 6, None)], 6: [("v", 7, None)],
                    7: [("qk", 3, None)],
                    8: [("v", 8, None)], 9: [("v", 9, None)],
                    10: [("v", 10, None)], 11: [("v", 11, None)],
                    12: [("v", 12, None)], 13: [("v", 13, None)],
                    14: [("v", 14, None)], 15: [("v", 15, None)]}
        # tiles 1-3: previous tile's spill attnV pairs + finish steps (spread
        # out so each step's engine work has slack before its consumer)
        TN_EXTRA = {0: [("spill", 10)], 1: [("spill", 11)],
                    2: [("spill", 12)], 3: [("spill", 13)],
                    4: [("spill", 14)], 5: [("spill", 15)],
                    6: [("fina",)], 7: [("finb",)],
                    9: [("finc",)], 11: [("find",)]}
        AUX = ((aux_ps, "bcq"), (aux_ps, "fpq"))

        emit_qk_slice(0, AUX)
        for n in range(4):
            for p in range(16):
                ps = emit_scores(n, p)
                if p >= LAGS[n]:
                    emit_attnv(n, p - LAGS[n])
                if n == 0:
                    for item in T0_EXTRA.get(p, []):
                        if item[0] == "qk":
                            emit_qk_slice(item[1], item[2] or AUX)
                        else:
                            emit_v(item[1], item[2])
                else:
                    for item in TN_EXTRA.get(p, []):
                        if item[0] == "spill":
                            emit_attnv(n - 1, item[1])
                        elif item[0] == "fina":
                            fin_a(n - 1)
                        elif item[0] == "finb":
                            fin_b(n - 1)
                        elif item[0] == "finc":
                            fin_c(n - 1)
                        else:
                            fin_d(n - 1)
                emit_exp(n, p, ps)
        for p in range(16 - LAGS[3], 16):
            emit_attnv(3, p)
        fin_a(3)
        fin_b(3)
        fin_c(3)
        fin_d(3)

    nc.compile()
    return nc


def host_prep(x, gamma, beta, Wq, bq, Wk, bk, Wv, bv, Wo, bo):
    """Build the 8 per-core input dicts."""
    f32 = lambda a: np.ascontiguousarray(np.asarray(a, np.float32))
    x = f32(x)
    gamma, beta = f32(gamma), f32(beta)
    Wq, Wk, Wv, Wo = f32(Wq), f32(Wk), f32(Wv), f32(Wo)
    bq, bk, bv, bo = f32(bq), f32(bk), f32(bv), f32(bo)

    wq_dup = np.ascontiguousarray(np.concatenate([Wq, Wq], axis=1))
    z = np.zeros((64, 64), np.float32)
    wk_blk = np.ascontiguousarray(np.block([[Wk, z], [z, Wk]]))
    wv_blk = np.ascontiguousarray(np.block([[Wv, z], [z, Wv]]))
    comb = np.zeros((128, 128), np.float32)
    comb[:64, :64] = 1.0 / 64.0
    comb[64:, 64:] = 1.0 / 64.0
    bo_f = bv @ Wo + bo  # fold v bias through the out-projection
    mdt_np = mybir.dt.np(MDT)
    m = lambda a: np.ascontiguousarray(a).astype(mdt_np)
    shared = {
        "wq": m(wq_dup), "wk": m(wk_blk), "wv": m(wv_blk), "wo": m(Wo),
        "bq": np.ascontiguousarray(np.tile(bq, 2)[:, None]),
        "bo": np.ascontiguousarray(
            np.concatenate([bo_f, np.zeros(64, np.float32)])[:, None]),
        "gam": np.ascontiguousarray(np.tile(gamma, 2)[:, None]),
        "bet": np.ascontiguousarray(np.tile(beta, 2)[:, None]),
        "comb": comb,
    }
    in_maps = []
    for core in range(8):
        b, h = core // 2, core % 2
        xT = x[b].reshape(HW, C).T  # [64, 4096]
        halves = xT.reshape(C, 2, HALF)[:, [h, 1 - h], :]
        xp = np.ascontiguousarray(halves.transpose(1, 0, 2).reshape(128, HALF))
        in_maps.append({"xp": xp, **shared})
    return in_maps


def assemble(results, dtype):
    out = np.empty((B, HW, C), np.float32)
    for core in range(8):
        b, h = core // 2, core % 2
        out[b, HALF * h: HALF * h + HALF] = results[core]["out"].T
    return out.reshape(B, H, W, C).astype(dtype, copy=False)


_NC_CACHE = []


def kernel(x, gamma, beta, Wq, bq, Wk, bk, Wv, bv, Wo, bo):
    from concourse.bass_utils import run_bass_kernel_spmd

    if not _NC_CACHE:
        _NC_CACHE.append(build_nc())
    nc = _NC_CACHE[0]
    in_maps = host_prep(x, gamma, beta, Wq, bq, Wk, bk, Wv, bv, Wo, bo)
    res = run_bass_kernel_spmd(nc, in_maps, core_ids=list(range(8)))
    return assemble(res.results, np.asarray(x).dtype)


if __name__ == "__main__":
    rng = np.random.default_rng(0)
    inputs = {
        "x": rng.standard_normal((B, H, W, C)).astype(np.float32),
        "gamma": np.ones(C, np.float32), "beta": np.zeros(C, np.float32),
        "Wq": (rng.standard_normal((C, C)) / 8).astype(np.float32),
        "bq": np.zeros(C, np.float32),
        "Wk": (rng.standard_normal((C, C)) / 8).astype(np.float32),
        "bk": np.zeros(C, np.float32),
        "Wv": (rng.standard_normal((C, C)) / 8).astype(np.float32),
        "bv": np.zeros(C, np.float32),
        "Wo": (rng.standard_normal((C, C)) / 8).astype(np.float32),
        "bo": np.zeros(C, np.float32),
    }
    out = kernel(**inputs)
    print("kernel ran, out shape", out.shape, out.dtype)


# revision 57
# speedup vs baseline: 1.1641x; 1.0043x over previous
"""Trainium2 Bass kernel for nn_AttentionBlock (B=4, H=W=64, C=64, GroupNorm(8) +
full spatial self-attention), distributed over 8 NeuronCores.

Sharding: core i handles batch b=i//2 and query-half h=i%2 (2048 of the 4096
spatial positions). Each core computes the full GroupNorm and K/V for its
image (cheap) and attention only for its query half. No collectives.

v2 pipeline:
- exp split across THREE engines: ACT (table exp) for some score pairs,
  Pool (gpsimd) and DVE for the rest via a single-op int16 Schraudolph
  (i16 = s*23.083 + 16256.5 truncated, bitcast bf16 ~= e^(s/8), max rel err
  ~4%, final output err ~3e-3; denominator uses the same approximated
  weights so softmax normalization stays consistent).
- PE stream is gap-free: warmup matmuls ramp the clock during GroupNorm
  stats, then per tile scores-pair p / attnV pair p-3 alternate, with
  qk/v production and prev-tile finish matmuls slotted into the bubbles.
- biases: bq/bk folded into the q/k PSUM->SBUF copies (per-partition add);
  bv folded into bo on the host (bv @ Wo + bo).
- softmax denominators ride as a 65th ones-column of V; reciprocal via
  the fast custom-DVE op on [1,512] then broadcast by a bf16 PE matmul.
"""

import sys

sys.path.insert(0, "/opt/trn_rl_repo")

import numpy as np

import concourse.bacc as bacc
import concourse.tile as tile
from concourse import mybir

B, H, W, C = 4, 64, 64, 64
HW = H * W  # 4096
HALF = HW // 2  # 2048
EPS = 1e-5
SCALE = C ** -0.5

F32 = mybir.dt.float32
MDT = mybir.dt.bfloat16  # PE matmul operand dtype (scores/projections)
I16 = mybir.dt.int16
I8 = mybir.dt.int8
F8E5 = mybir.dt.float8e5  # attn weights (e5m2: range to 57344 covers e^9)
F8E4 = mybir.dt.float8e4  # v values (e4m3)

# Schraudolph exp in bf16-bit space: i16 = round(s * 2^7/ln2 * SCALE + 127*2^7)
SCH_SCALE = float((2.0 ** 7) / np.log(2.0) * SCALE)
SCH_BIAS = 16251.0  # 127*2^7 shifted -5.5 to center the one-sided
# mantissa-interpolation error (+0..6.7%) around zero
# same trick in e5m2-bit space: i8 = s * 2^2/ln2 * SCALE + 15*2^2
SCH8_SCALE = float(4.0 / np.log(2.0) * SCALE)
SCH8_BIAS = 60.25

NWARM = 32  # PE warmup matmuls (ramp p-state during GN stats,
# and bridge the stats-chain tail so the HAM never sees a >3.4us idle)
LAGS = [6, 6, 6, 3]  # attnV trails scores by LAG pairs; short last tile
# so the post-loop drain is small

# engine per exp pair: A=ACT table exp, D=DVE int16-schraudolph. (Pool cannot
# read PSUM on TRN2, so it only gets SBUF->SBUF work: xn, recb, final out.)
# Tile tails lean A so DVE is clear for the next tile's start.
EMAP0 = ['D', 'A', 'D', 'A', 'D', 'A', 'D', 'A',
         'D', 'A', 'D', 'A', 'A', 'D', 'A', 'A']      # A9 D7
EMAPN = ['D', 'A', 'D', 'A', 'D', 'A', 'D', 'A',
         'D', 'D', 'A', 'D', 'D', 'A', 'A', 'A']      # A8 D8
EMAP3 = ['D', 'A'] * 7 + ['A', 'D']  # strict alternation: tile 3 runs LAG=3, so each
# exp must land within ~2 pairs of its scores
EMAPS = [EMAP0, EMAPN, EMAPN, EMAP3]


def build_nc():
    nc = bacc.Bacc("TRN2", debug=False, num_devices=8)

    # ---- DRAM I/O ----
    xp_d = nc.dram_tensor("xp", [128, HALF], F32, kind="ExternalInput")
    wq_d = nc.dram_tensor("wq", [64, 128], MDT, kind="ExternalInput")
    wk_d = nc.dram_tensor("wk", [128, 128], MDT, kind="ExternalInput")
    wv_d = nc.dram_tensor("wv", [128, 128], MDT, kind="ExternalInput")
    wo_d = nc.dram_tensor("wo", [64, 64], MDT, kind="ExternalInput")
    bq_d = nc.dram_tensor("bq", [128, 1], F32, kind="ExternalInput")
    bo_d = nc.dram_tensor("bo", [128, 1], F32, kind="ExternalInput")
    gam_d = nc.dram_tensor("gam", [128, 1], F32, kind="ExternalInput")
    bet_d = nc.dram_tensor("bet", [128, 1], F32, kind="ExternalInput")
    comb_d = nc.dram_tensor("comb", [128, 128], F32, kind="ExternalInput")
    out_d = nc.dram_tensor("out", [64, HALF], F32, kind="ExternalOutput")

    with tile.TileContext(nc) as tc, \
         tc.tile_pool(name="singles", bufs=1) as singles, \
         tc.tile_pool(name="stats", bufs=1) as stats, \
         tc.tile_pool(name="sc_ps", bufs=2, space="PSUM") as sc_ps, \
         tc.tile_pool(name="pacc_ps", bufs=2, space="PSUM") as pacc_ps, \
         tc.tile_pool(name="aux_ps", bufs=1, space="PSUM") as aux_ps, \
         tc.tile_pool(name="work", bufs=2) as work:

        # ---- input DMAs: everything on the sync hwdge queue (the issuing
        # engine is otherwise idle; putting DMAs on the ACT queue would
        # block the sqrt/exp table work behind descriptor writes) ----
        x_sb = singles.tile([128, HALF], F32)
        for r in range(4):
            nc.sync.dma_start(
                x_sb[:, 512 * r: 512 * r + 512],
                xp_d.ap()[:, 512 * r: 512 * r + 512],
            )
        gam_sb = singles.tile([128, 1], F32)
        nc.sync.dma_start(gam_sb[:], gam_d.ap())
        bet_sb = singles.tile([128, 1], F32)
        nc.sync.dma_start(bet_sb[:], bet_d.ap())
        comb_sb = singles.tile([128, 128], F32)
        nc.sync.dma_start(comb_sb[:], comb_d.ap())
        wk_sb = singles.tile([128, 128], MDT)
        nc.sync.dma_start(wk_sb[:], wk_d.ap())
        wq_sb = singles.tile([64, 128], MDT)
        nc.sync.dma_start(wq_sb[:], wq_d.ap())
        bq_sb = singles.tile([128, 1], F32)
        nc.sync.dma_start(bq_sb[:], bq_d.ap())
        wv_sb = singles.tile([128, 128], MDT)
        nc.sync.dma_start(wv_sb[:], wv_d.ap())
        wo_sb = singles.tile([64, 64], MDT)
        nc.sync.dma_start(wo_sb[:], wo_d.ap())
        bo_sb = singles.tile([128, 1], F32)
        nc.sync.dma_start(bo_sb[:], bo_d.ap())

        # ---- big SBUF tensors ----
        xn_r = singles.tile([128, HALF], MDT)
        q_dup = singles.tile([128, HALF], MDT)
        kt_sb = singles.tile([128, HALF], MDT)
        v_all = singles.tile([128, 65 * 32], MDT)
        attnexp = singles.tile([128, 1024 * 16], MDT)
        out_sb = singles.tile([64, HALF], F32)
        ones_sb = singles.tile([128, 512], MDT)

        # constants on Pool, first thing (warmup matmuls read ones_sb)
        nc.gpsimd.memset(ones_sb[:], 1.0)
        v4 = v_all[:].rearrange("p (h t e) -> p h t e", h=2, e=65)
        nc.gpsimd.memset(v4[:, :, :, 64:65], 1.0)

        # pre-warm the sqrt ACT table set (used by the GN rstd); the exp
        # table is loaded right after the single sqrt below
        scr = stats.tile([128, 1], F32)
        nc.vector.memset(scr[:], 1.0)
        nc.scalar.activation(scr[:], scr[:], mybir.ActivationFunctionType.Sqrt)

        # ---- PE warmup: ramp the activity monitor while DVE does GN stats
        # (rides the sc-tag banks, which are free until the first scores) ----
        for w in range(NWARM):
            wps = sc_ps.tile([128, 512], F32, tag="sc", name=f"warm{w}")
            nc.tensor.matmul(wps[:], ones_sb[:, 0:128], ones_sb[:, :],
                             start=True, stop=True)

        # ---- GroupNorm stats: bn per partition per 512-slice, then a
        # block-diagonal averaging matmul combines across channels ----
        st6 = stats.tile([128, 4, 6], F32)
        mv4 = stats.tile([128, 4, 2], F32)
        for r in range(4):
            nc.vector.bn_stats(st6[:, r, :], x_sb[:, 512 * r: 512 * r + 512])
            nc.vector.bn_aggr(mv4[:, r, :], st6[:, r, :])
        smat = stats.tile([128, 8], F32)  # cols 0-3 mean, 4-7 E[x^2]
        nc.vector.tensor_copy(smat[:, 0:4], mv4[:, :, 0])
        nc.vector.tensor_mul(smat[:, 4:8], mv4[:, :, 0], mv4[:, :, 0])
        nc.vector.tensor_add(smat[:, 4:8], smat[:, 4:8], mv4[:, :, 1])

        cps = pacc_ps.tile([128, 8], F32, tag="pacc")
        nc.tensor.matmul(cps[:], comb_sb[:], smat[:], start=True, stop=True)
        gstat = stats.tile([128, 8], F32)  # 0-3 mean_g, 4-7 E2_g
        nc.vector.tensor_copy(gstat[:], cps[:])

        # var = E2 - mean^2; EPS folds into the sqrt's activation bias.
        # rstd = 1/sqrt(var+EPS) via ACT sqrt + fast custom-DVE reciprocal
        # (the sanctioned accurate path; far fewer serial ops than a
        # bit-trick Newton chain)
        ve = stats.tile([128, 4], F32)
        nc.vector.tensor_mul(ve[:], gstat[:, 0:4], gstat[:, 0:4])
        nc.vector.tensor_sub(ve[:], gstat[:, 4:8], ve[:])
        eps_sb = stats.tile([128, 1], F32)
        nc.vector.memset(eps_sb[:], EPS)
        sve = stats.tile([128, 4], F32)
        nc.scalar.activation(sve[:], ve[:],
                             mybir.ActivationFunctionType.Sqrt,
                             bias=eps_sb[:])
        # switch ACT to the exp table now, during idle time
        nc.scalar.activation(scr[:], scr[:], mybir.ActivationFunctionType.Exp)
        rstd = stats.tile([128, 4], F32)
        nc.vector.reciprocal_approx_fast(out=rstd[:], in_=sve[:])

        gsc = stats.tile([128, 4], F32)
        nc.vector.tensor_scalar_mul(gsc[:], rstd[:], gam_sb[:])
        gbias = stats.tile([128, 4], F32)
        nc.vector.tensor_mul(gbias[:], gstat[:, 0:4], gsc[:])
        nc.vector.tensor_scalar(
            out=gbias[:], in0=gbias[:], scalar1=-1.0, scalar2=bet_sb[:],
            op0=mybir.AluOpType.mult, op1=mybir.AluOpType.add,
        )
        # the fp32 residual pass folds in bo (bo rides rows 0:63 of the bias;
        # rows 64:127 of x_sb are never read again after the qkv matmuls)
        gbias2 = stats.tile([128, 4], F32)
        nc.vector.tensor_scalar_add(gbias2[:], gbias[:], bo_sb[:])
        # xn = x * gsc + gbias: slice 0's bf16 copy runs on DVE (it gates
        # qk0 -> first scores); the rest and the fp32 residual pass run on
        # the slow-but-idle Pool. Per slice: bf16 read first, then the
        # in-place fp32 overwrite (Pool ops ordered; DVE xn0 emitted before
        # Pool's slice-0 overwrite so the framework serializes the WAR).
        nc.vector.tensor_scalar(
            out=xn_r[:, 0:512], in0=x_sb[:, 0:512],
            scalar1=gsc[:, 0:1], scalar2=gbias[:, 0:1],
            op0=mybir.AluOpType.mult, op1=mybir.AluOpType.add,
        )
        for r in range(4):
            sl = slice(512 * r, 512 * r + 512)
            if r > 0:
                nc.gpsimd.tensor_scalar(
                    out=xn_r[:, sl], in0=x_sb[:, sl],
                    scalar1=gsc[:, r: r + 1], scalar2=gbias[:, r: r + 1],
                    op0=mybir.AluOpType.mult, op1=mybir.AluOpType.add,
                )
            nc.gpsimd.tensor_scalar(
                out=x_sb[:, sl], in0=x_sb[:, sl],
                scalar1=gsc[:, r: r + 1], scalar2=gbias2[:, r: r + 1],
                op0=mybir.AluOpType.mult, op1=mybir.AluOpType.add,
            )

        # ---- emission helpers ----
        def emit_qk_slice(t, pool_tags):
            # k^T packed by half (lhsT = blockdiag(Wk, Wk)); q^T duplicated on
            # both partition halves (lhsT = [Wq | Wq]). bk is dropped exactly
            # (a per-query score constant cancels in softmax); bq folds into
            # the q copy as a per-partition bias on ACT.
            sl = slice(512 * t, 512 * t + 512)
            pool_k, tag_k = pool_tags[0]
            pool_q, tag_q = pool_tags[1]
            ps2 = pool_k.tile([128, 512], F32, tag=tag_k, name=f"kps{t}")
            nc.tensor.matmul(ps2[:], wk_sb[:], xn_r[:, sl], start=True,
                             stop=True)
            nc.scalar.activation(kt_sb[:, sl], ps2[:],
                                 mybir.ActivationFunctionType.Identity)
            ps = pool_q.tile([128, 512], F32, tag=tag_q, name=f"qps{t}")
            nc.tensor.matmul(ps[:], wq_sb[:], xn_r[0:64, sl], start=True,
                             stop=True)
            nc.scalar.activation(
                q_dup[:, sl], ps[:], mybir.ActivationFunctionType.Identity,
                bias=bq_sb[:],
            )

        def emit_v(u, pool_tag=None):
            # v position-major, two 128-position chunks (halves) per matmul;
            # single copy with a dual-chunk strided output AP. Even u on the
            # bcq psum tag + DVE copy, odd u on fpq + ACT.
            sl = slice(128 * u, 128 * u + 128)
            pool, tag = pool_tag or (aux_ps, "bcq" if u % 2 == 0 else "fpq")
            ps = pool.tile([128, 128], F32, tag=tag, name=f"vps{u}")
            nc.tensor.matmul(ps[:], xn_r[:, sl], wv_sb[:], start=True,
                             stop=True)
            psr = ps[:].rearrange("p (h e) -> p h e", h=2)
            nc.vector.tensor_copy(v4[:, :, u, 0:64], psr[:, :, :])

        def emit_scores(n, p):
            # pair p: kv chunks p (half0, PE rows 0-63) and p+16 (half1, rows
            # 64-127) run concurrently; one [128,1024] 2-bank psum tile
            qsl = slice(512 * n, 512 * n + 512)
            ksl = slice(128 * p, 128 * p + 128)
            ps = sc_ps.tile([128, 1024], F32, tag="sc", name=f"sc{n}_{p}")
            nc.tensor.matmul(ps[:, 0:512], kt_sb[0:64, ksl],
                             q_dup[0:64, qsl], start=True, stop=True)
            nc.tensor.matmul(ps[:, 512:1024], kt_sb[64:128, ksl],
                             q_dup[64:128, qsl], start=True, stop=True)
            return ps

        def emit_exp(n, p, ps):
            # attnexp layout pair-major: chunk p at 1024p, chunk p+16 at
            # 1024p+512 — both written by this single instruction
            dst = attnexp[:, 1024 * p: 1024 * p + 1024]
            e = EMAPS[n][p]
            if e == 'A':
                nc.scalar.activation(dst, ps[:],
                                     mybir.ActivationFunctionType.Exp,
                                     scale=SCALE)
            else:
                nc.vector.tensor_scalar(
                    out=dst.bitcast(I16), in0=ps[:],
                    scalar1=SCH_SCALE, scalar2=SCH_BIAS,
                    op0=mybir.AluOpType.mult, op1=mybir.AluOpType.add,
                )

        paccs = {}

        def emit_attnv(n, p):
            # kv chunk pair (p, p+16) — consumes exp pair p. One fp8
            # DoubleRow matmul per pair: contraction 2x128 kv, 0.5 cyc/row.
            if n not in paccs:
                paccs[n] = pacc_ps.tile([65, 512], F32, tag="pacc",
                                        name=f"pacc{n}")
            pacc = paccs[n]
            for t in (p, p + 16):
                off = 1024 * p + (512 if t >= 16 else 0)
                nc.tensor.matmul(
                    pacc[:], v_all[:, 65 * t: 65 * t + 65],
                    attnexp[:, off: off + 512],
                    start=(t == 0), stop=(t == 31),
                )

        # finish chain for tile n, split into steps scheduled across pairs of
        # tile n+1 so the PE stream stays dense
        fin = {}

        def fin_a(n):
            # free the PSUM accumulator ASAP: unnormalized proj rows (bf16)
            # on ACT; the raw denominator row stays in PSUM for fin_b's rec
            pacc = paccs[n]
            projn_u = work.tile([64, 512], MDT, tag="projn", name=f"pn{n}")
            nc.scalar.activation(projn_u[:], pacc[0:64, :],
                                 mybir.ActivationFunctionType.Identity)
            fin[n] = (projn_u,)

        def fin_b(n):
            # per-query 1/denom straight off PSUM (fast custom-DVE approx).
            # The custom op ignores input partition offsets, so run it over
            # all 65 partitions (same cost: DVE time = free size) and use
            # row 64. Then a tiny bf16 convert on Pool for the broadcast.
            pacc = paccs.pop(n)
            (projn_u,) = fin[n]
            rec = work.tile([65, 512], F32, tag="rec", name=f"rec{n}")
            nc.vector.reciprocal_approx_fast(out=rec[:], in_=pacc[:, :])
            recb = work.tile([1, 512], MDT, tag="recb", name=f"recb{n}")
            nc.scalar.activation(recb[:], rec[64:65, :],
                                 mybir.ActivationFunctionType.Identity)
            fin[n] = (projn_u, recb)

        def fin_c(n):
            # PE: broadcast 1/denom to [64,512] + out-projection
            projn_u, recb = fin[n]
            bc_ps = aux_ps.tile([64, 512], F32, tag="bcq", name=f"bc{n}")
            nc.tensor.matmul(bc_ps[:], ones_sb[0:1, 0:64], recb[:],
                             start=True, stop=True)
            fps = aux_ps.tile([64, 512], F32, tag="fpq", name=f"fps{n}")
            nc.tensor.matmul(fps[:], wo_sb[:], projn_u[:], start=True,
                             stop=True)
            bc_sb = work.tile([64, 512], F32, tag="bc", name=f"bcs{n}")
            nc.scalar.activation(bc_sb[:], bc_ps[:],
                                 mybir.ActivationFunctionType.Identity)
            fin[n] = (fps, bc_sb)

        def fin_d(n):
            # normalize on DVE, then bias + residual + store on Pool
            fps, bc_sb = fin.pop(n)
            qsl = slice(512 * n, 512 * n + 512)
            mn = work.tile([64, 512], F32, tag="mn", name=f"mn{n}")
            nc.vector.tensor_mul(mn[:], fps[:], bc_sb[:])
            # last tile's residual-add on DVE: it is the serial kernel tail
            eng = nc.vector if n == 3 else nc.gpsimd
            eng.tensor_add(out_sb[:, qsl], mn[:], x_sb[0:64, qsl])
            nc.sync.dma_start(out_d.ap()[:, qsl], out_sb[:, qsl])

        # ---- software-pipelined attention ----
        # tile 0 extras: qk slices 1-3 and v chunks produced just in time
        # (scores pair p needs kt slice p//4, attnV pair p-LAG needs v chunk
        # p-LAG). The earliest qkv psums ride the pacc-tag banks (free until
        # the first pacc allocation at p=LAG); the rest alternate bcq/fpq so
        # every tenant's copy has >= 2 pairs to drain before bank reuse.
        PACC_TAG = (pacc_ps, "pacc")
        T0_EXTRA = {0: [("qk", 1, (PACC_TAG, PACC_TAG))],
                    1: [("v", 0, PACC_TAG), ("v", 1, PACC_TAG)],
                    2: [("v", 2, None), ("v", 3, None)],
                    3: [("qk", 2, None)],
                    4: [("v", 4, None), ("v", 5, None)],
                    5: [("v", 6, None)], 6: [("v", 7, None)],
                    7: [("qk", 3, None)],
                    8: [("v", 8, None)], 9: [("v", 9, None)],
                    10: [("v", 10, None)], 11: [("v", 11, None)],
                    12: [("v", 12, None)], 13: [("v", 13, None)],
                    14: [("v", 14, None)], 15: [("v", 15, None)]}
        # tiles 1-3: previous tile's spill attnV pairs + finish steps (spread
        # out so each step's engine work has slack before its consumer)
        TN_EXTRA = {0: [("spill", 10)], 1: [("spill", 11)],
                    2: [("spill", 12)], 3: [("spill", 13)],
                    4: [("spill", 14)], 5: [("spill", 15)],
                    6: [("fina",)], 7: [("finb",)],
                    9: [("finc",)], 11: [("find",)]}
        AUX = ((aux_ps, "bcq"), (aux_ps, "fpq"))

        emit_qk_slice(0, AUX)
        for n in range(4):
            for p in range(16):
                ps = emit_scores(n, p)
                if p >= LAGS[n]:
                    emit_attnv(n, p - LAGS[n])
                if n == 0:
                    for item in T0_EXTRA.get(p, []):
                        if item[0] == "qk":
                            emit_qk_slice(item[1], item[2] or AUX)
                        else:
                            emit_v(item[1], item[2])
                else:
                    for item in TN_EXTRA.get(p, []):
                        if item[0] == "spill":
                            emit_attnv(n - 1, item[1])
                        elif item[0] == "fina":
                            fin_a(n - 1)
                        elif item[0] == "finb":
                            fin_b(n - 1)
                        elif item[0] == "finc":
                            fin_c(n - 1)
                        else:
                            fin_d(n - 1)
                emit_exp(n, p, ps)
        for p in range(16 - LAGS[3], 16):
            emit_attnv(3, p)
        fin_a(3)
        fin_b(3)
        fin_c(3)
        fin_d(3)

    nc.compile()
    return nc


def host_prep(x, gamma, beta, Wq, bq, Wk, bk, Wv, bv, Wo, bo):
    """Build the 8 per-core input dicts."""
    f32 = lambda a: np.ascontiguousarray(np.asarray(a, np.float32))
    x = f32(x)
    gamma, beta = f32(gamma), f32(beta)
    Wq, Wk, Wv, Wo = f32(Wq), f32(Wk), f32(Wv), f32(Wo)
    bq, bk, bv, bo = f32(bq), f32(bk), f32(bv), f32(bo)

    wq_dup = np.ascontiguousarray(np.concatenate([Wq, Wq], axis=1))
    z = np.zeros((64, 64), np.float32)
    wk_blk = np.ascontiguousarray(np.block([[Wk, z], [z, Wk]]))
    wv_blk = np.ascontiguousarray(np.block([[Wv, z], [z, Wv]]))
    comb = np.zeros((128, 128), np.float32)
    comb[:64, :64] = 1.0 / 64.0
    comb[64:, 64:] = 1.0 / 64.0
    bo_f = bv @ Wo + bo  # fold v bias through the out-projection
    mdt_np = mybir.dt.np(MDT)
    m = lambda a: np.ascontiguousarray(a).astype(mdt_np)
    shared = {
        "wq": m(wq_dup), "wk": m(wk_blk), "wv": m(wv_blk), "wo": m(Wo),
        "bq": np.ascontiguousarray(np.tile(bq, 2)[:, None]),
        "bo": np.ascontiguousarray(
            np.concatenate([bo_f, np.zeros(64, np.float32)])[:, None]),
        "gam": np.ascontiguousarray(np.tile(gamma, 2)[:, None]),
        "bet": np.ascontiguousarray(np.tile(beta, 2)[:, None]),
        "comb": comb,
    }
    in_maps = []
    for core in range(8):
        b, h = core // 2, core % 2
        xT = x[b].reshape(HW, C).T  # [64, 4096]
        halves = xT.reshape(C, 2, HALF)[:, [h, 1 - h], :]
        xp = np.ascontiguousarray(halves.transpose(1, 0, 2).reshape(128, HALF))
        in_maps.append({"xp": xp, **shared})
    return in_maps


def assemble(results, dtype):
    out = np.empty((B, HW, C), np.float32)
    for core in range(8):
        b, h = core // 2, core % 2
        out[b, HALF * h: HALF * h + HALF] = results[core]["out"].T
    return out.reshape(B, H, W, C).astype(dtype, copy=False)


_NC_CACHE = []


def kernel(x, gamma, beta, Wq, bq, Wk, bk, Wv, bv, Wo, bo):
    from concourse.bass_utils import run_bass_kernel_spmd

    if not _NC_CACHE:
        _NC_CACHE.append(build_nc())
    nc = _NC_CACHE[0]
    in_maps = host_prep(x, gamma, beta, Wq, bq, Wk, bk, Wv, bv, Wo, bo)
    res = run_bass_kernel_spmd(nc, in_maps, core_ids=list(range(8)))
    return assemble(res.results, np.asarray(x).dtype)


if __name__ == "__main__":
    rng = np.random.default_rng(0)
    inputs = {
        "x": rng.standard_normal((B, H, W, C)).astype(np.float32),
        "gamma": np.ones(C, np.float32), "beta": np.zeros(C, np.float32),
        "Wq": (rng.standard_normal((C, C)) / 8).astype(np.float32),
        "bq": np.zeros(C, np.float32),
        "Wk": (rng.standard_normal((C, C)) / 8).astype(np.float32),
        "bk": np.zeros(C, np.float32),
        "Wv": (rng.standard_normal((C, C)) / 8).astype(np.float32),
        "bv": np.zeros(C, np.float32),
        "Wo": (rng.standard_normal((C, C)) / 8).astype(np.float32),
        "bo": np.zeros(C, np.float32),
    }
    out = kernel(**inputs)
    print("kernel ran, out shape", out.shape, out.dtype)
